# revision 39
# baseline (speedup 1.0000x reference)
"""SwiGLU-projected causal MHA (B=4, S=2048, D=1024, H=16) on 8 TRN2 NeuronCores.

Sharding: core c -> (batch b = c//2, head-group g = c%2).  Each core computes
the SwiGLU Q/K/V projections for its 512 output channels (= 8 heads) of its
batch, runs causal attention for those heads, and produces a partial output
projection (contraction over its 512 channels).  The host sums the two
partials per batch and adds the output bias.

Key structure (v2):
  - Every matmul is 128x128 mode (K padded to 128 via zeroed weight halves,
    bias matmuls padded with zero rows) -> zero PE mode-switch drains.
  - K tensor stored per-head-parity zero-padded (ktz_e/ktz_o) so score
    matmuls contract K=128 with the other head's rows zeroed.
  - exp batched over [128, 2, 512] PSUM pairs (both heads of a pair per kc).
  - softmax denominator comes out of the AV matmul (ones column in V);
    reciprocal via DVE reciprocal_approx_fast; broadcast across partitions
    via one K=128 matmul with a constant indicator matrix.
  - Phase A (projections) and phase B (attention) are interleaved in
    emission order, paced by estimated PE cost, so exp/DVE work hides under
    projection matmuls and the PE never idles:
      region0: A(v0,k0,q0)
      region1: A(v1,k1,q1) || B(qg0)
      region2: A(v2,k2,q2) || B(qg1)
      region3: A(v3,k3,q3) || B(qg2)   (qg2 only needs t<=2 K/V)
      tail:    B(qg3)
  - PSUM: A pool 2 banks, scores 2x[128,2,512]=4 banks, ctx [128,2,512]=2.
    bc/out-proj tiles borrow the score pool.  Total exactly 8 banks.
"""
import sys

sys.path.insert(0, "/opt/trn_rl_repo")
import numpy as np

import concourse.bacc as bacc
import concourse.tile as tile
import concourse.mybir as mybir

B, S, D = 4, 2048, 1024
H, DK = 16, 64
NCORES = 8
GCH = 512          # channels per core (8 heads)
NT = S // 128      # 16 seq chunks
F32 = mybir.dt.float32
F32R = mybir.dt.float32r
ACTF = mybir.ActivationFunctionType
ALU = mybir.AluOpType

TRACE = False          # set by test.py for profiling runs
TRACE_CORES = None
LAST_RESULT = None     # BassKernelResults stash for test.py
MM_DTYPE = "bf16"
DEBUG = False          # adds intermediate-dump DRAM outputs


def build_program(mask_mode):
    """mask_mode: 'causal' (tril), 'full' (all ones), 'general' (arbitrary)."""
    MMD = mybir.dt.bfloat16 if MM_DTYPE == "bf16" else F32R
    nc = bacc.Bacc("TRN2", target_bir_lowering=False, debug=False)

    xT = {s: nc.dram_tensor(f"x{s}T", [D, S], MMD, kind="ExternalInput")
          for s in "qkv"}
    w1T = {s: nc.dram_tensor(f"w1T_{s}", [D, GCH], MMD, kind="ExternalInput")
           for s in "qkv"}
    w2T = {s: nc.dram_tensor(f"w2T_{s}", [D, GCH], MMD, kind="ExternalInput")
           for s in "qkv"}
    bias_d = {}
    for s in "qk":
        for bn in ("b1", "b2", "b1h"):
            bias_d[f"{bn}_{s}"] = nc.dram_tensor(f"{bn}_{s}", [128, 4], F32,
                                                 kind="ExternalInput")
    b1v_d = nc.dram_tensor("b1_v", [1, GCH], MMD, kind="ExternalInput")
    b2v_d = nc.dram_tensor("b2_v", [1, GCH], MMD, kind="ExternalInput")
    woT_d = nc.dram_tensor("woT", [128, 4, D], MMD, kind="ExternalInput")
    pat_d = m01T_d = None
    if mask_mode == "causal":
        pat_d = nc.dram_tensor("pat", [128, 4, 512], MMD, kind="ExternalInput")
    elif mask_mode == "general":
        m01T_d = nc.dram_tensor("m01T", [S, S], MMD, kind="ExternalInput")
    pout_d = nc.dram_tensor("pout", [S, D], F32, kind="ExternalOutput")
    dbg = {}
    if DEBUG:
        for nm, shp in (("qt", [128, 4, S]), ("kt0", [128, 4, S]),
                        ("kt1", [128, 4, S]), ("v", [128, NT, 8, 65]),
                        ("attn", [128, 2, 512]), ("ct", [128, 4, 4, 512]),
                        ("rec", [128, 512]), ("bcw", [128, 128]),
                        ("bcsb", [128, 512])):
            dbg[nm] = nc.dram_tensor(f"dbg_{nm}", shp, MMD,
                                     kind="ExternalOutput")

    def kc_count(qg):
        return 4 * qg + 4 if mask_mode == "causal" else NT

    with tile.TileContext(nc) as tc:
        with (
            tc.tile_pool(name="persist", bufs=1) as persist,
            tc.tile_pool(name="xpool", bufs=26) as xpool,
            tc.tile_pool(name="stage", bufs=2) as stage,
            tc.tile_pool(name="apool", bufs=4) as apool,
            tc.tile_pool(name="ctpool", bufs=2) as ctpool,
            tc.tile_pool(name="cxspool", bufs=3) as cxspool,
            tc.tile_pool(name="smalls", bufs=2) as smalls,
            tc.tile_pool(name="ostage", bufs=3) as ostage,
            tc.tile_pool(name="mpool", bufs=2) as mpool,
            tc.tile_pool(name="pps", bufs=2, space="PSUM") as pps,
            tc.tile_pool(name="scps", bufs=2, space="PSUM") as scps,
            tc.tile_pool(name="cxps", bufs=1, space="PSUM") as cxps,
        ):
            # ---------------- persistent tiles + preloads ----------------
            qt_sb = persist.tile([128, 4, S], MMD, tag="qt")
            # zero-padded per-parity K: even heads live in partitions 0-63,
            # odd heads in 64-127; the other half stays zero so score
            # matmuls can contract K=128 (no PE mode switch).
            ktz = [persist.tile([128, 4, S], MMD, tag=f"ktz{p}",
                                name=f"ktz{p}")
                   for p in range(2)]
            v_sb = persist.tile([128, NT, 8, 65], MMD, tag="v")
            w1sb = {}
            w2sb = {}
            for s in "vkq":
                w1sb[s] = persist.tile([128, 8, GCH], MMD, tag=f"w1{s}",
                                       name=f"w1{s}")
                w2sb[s] = persist.tile([128, 8, GCH], MMD, tag=f"w2{s}",
                                       name=f"w2{s}")
            woT_sb = persist.tile([128, 4, D], MMD, tag="wo")

            # V biases padded to K=128 (row 0 = bias, rest zero), plus a
            # row-0-ones lhsT so the bias matmul is a full 128x128 matmul.
            b1v_sb = persist.tile([128, GCH], MMD, tag="b1v")
            b2v_sb = persist.tile([128, GCH], MMD, tag="b2v")
            onesrow = persist.tile([128, 128], MMD, tag="onesrow")
            # rec broadcast: bcW[0, 0:64] = 1, bcW[32, 64:128] = 1, rest 0.
            # bc = bcW.T @ rec_sb replicates rec rows 0/32 across partitions.
            bcW = persist.tile([128, 128], MMD, tag="bcW")
            rec_sb = persist.tile([128, 512], MMD, tag="rec")
            rec_f32 = persist.tile([33, 512], F32, tag="recf")
            den_pp = [persist.tile([33, 512], F32, tag=f"den{i}",
                                   name=f"den{i}")
                      for i in range(2)]
            nc.gpsimd.memset(den_pp[0][:], 1.0)
            nc.gpsimd.memset(den_pp[1][:], 1.0)

            nc.gpsimd.memset(ktz[0][:], 0.0)
            nc.gpsimd.memset(ktz[1][:], 0.0)
            nc.gpsimd.memset(b1v_sb[:], 0.0)
            nc.gpsimd.memset(b2v_sb[:], 0.0)
            nc.gpsimd.memset(onesrow[:], 0.0)
            nc.gpsimd.memset(onesrow[0:1, :], 1.0)
            nc.gpsimd.memset(bcW[:], 0.0)
            nc.gpsimd.memset(bcW[0:1, 0:64], 1.0)
            nc.gpsimd.memset(bcW[32:33, 64:128], 1.0)
            nc.gpsimd.memset(rec_sb[:], 0.0)
            nc.gpsimd.memset(rec_f32[:], 0.0)

            onescol = persist.tile([128, 1], F32, tag="onescol")
            nc.any.memset(onescol[:], 1.0)
            nc.vector.tensor_copy(
                v_sb[:, :, :, 64:65],
                onescol[:, None, :].to_broadcast([128, NT, 8, 1]),
            )

            # weight DMAs, v first (first A unit), per-dc chunks so the
            # first matmul can start after ~1.5us of DMA.
            nc.sync.dma_start(b1v_sb[0:1, :], b1v_d[:])
            nc.sync.dma_start(b2v_sb[0:1, :], b2v_d[:])
            for s in "vkq":
                for dc in range(8):
                    nc.sync.dma_start(
                        w1sb[s][:, dc, :],
                        w1T[s][dc * 128:(dc + 1) * 128, :])
                for dc in range(8):
                    nc.sync.dma_start(
                        w2sb[s][:, dc, :],
                        w2T[s][dc * 128:(dc + 1) * 128, :])
                if s != "v":
                    for bn in ("b1", "b2", "b1h"):
                        t_ = persist.tile([128, 4], F32, tag=f"{bn}{s}",
                                          name=f"{bn}{s}")
                        nc.sync.dma_start(t_[:], bias_d[f"{bn}_{s}"][:])
                        if bn == "b1":
                            b1sb = t_
                        elif bn == "b2":
                            b2sb = t_
                        else:
                            b1hsb = t_
                    if s == "k":
                        b1k, b2k, b1hk = b1sb, b2sb, b1hsb
                    else:
                        b1q, b2q, b1hq = b1sb, b2sb, b1hsb
            pat_sb = None
            if mask_mode == "causal":
                pat_sb = persist.tile([128, 4, 512], MMD, tag="pat")
                nc.sync.dma_start(pat_sb[:], pat_d[:])
            nc.sync.dma_start(woT_sb[:], woT_d[:])

            bias_qk = {"q": (b1q, b2q, b1hq), "k": (b1k, b2k, b1hk)}

            # ---------------- phase A unit emission ----------------
            xtiles = {}

            def prefetch_x(s, t):
                if (s, t) in xtiles:
                    return
                xs = []
                for dc in range(8):
                    xt = xpool.tile([128, 512], MMD, tag="xt")
                    nc.sync.dma_start(
                        xt[:],
                        xT[s][dc * 128:(dc + 1) * 128,
                              t * 512:(t + 1) * 512])
                    xs.append(xt)
                xtiles[(s, t)] = xs

            def a_subunit(s, t, jh, jj):
                """One (128 out-ch x 512 seq) SwiGLU tile: ps1 branch fully,
                stash 2*silu, then ps2 branch and write the destination."""
                j = jh * 2 + jj
                xts = xtiles[(s, t)]
                ps1 = pps.tile([128, 512], F32, tag="pp", name="ps1")
                for dc in range(8):
                    if s == "v":
                        nc.tensor.matmul(
                            ps1[:], xts[dc][:, j * 128:(j + 1) * 128],
                            w1sb[s][:, dc, :],
                            start=(dc == 0), stop=False)
                    else:
                        nc.tensor.matmul(
                            ps1[:], w1sb[s][:, dc, j * 128:(j + 1) * 128],
                            xts[dc][:],
                            start=(dc == 0), stop=(dc == 7))
                if s == "v":
                    nc.tensor.matmul(ps1[:], onesrow[:], b1v_sb[:],
                                     start=False, stop=True)
                act = stage.tile([128, 512], F32, tag="act")
                silu = stage.tile([128, 512], F32, tag="silu")
                if s == "v":
                    # act = tanh(ps1/2); silu2 = ps1*(1+act) = 2*silu(ps1)
                    nc.scalar.activation(act[:], ps1[:], ACTF.Tanh, scale=0.5)
                    nc.vector.scalar_tensor_tensor(
                        silu[:], act[:], 1.0, ps1[:],
                        op0=ALU.add, op1=ALU.mult)
                else:
                    b1_, b2_, b1h_ = bias_qk[s]
                    nc.scalar.activation(
                        act[:], ps1[:], ACTF.Tanh,
                        scale=0.5, bias=b1h_[:, j:j + 1])
                    a_sb = stage.tile([128, 512], F32, tag="asb")
                    nc.vector.tensor_scalar_add(a_sb[:], ps1[:],
                                                b1_[:, j:j + 1])
                    nc.vector.scalar_tensor_tensor(
                        silu[:], act[:], 1.0, a_sb[:],
                        op0=ALU.add, op1=ALU.mult)
                ps2 = pps.tile([128, 512], F32, tag="pp", name="ps2")
                for dc in range(8):
                    if s == "v":
                        nc.tensor.matmul(
                            ps2[:], xts[dc][:, j * 128:(j + 1) * 128],
                            w2sb[s][:, dc, :],
                            start=(dc == 0), stop=False)
                    else:
                        nc.tensor.matmul(
                            ps2[:], w2sb[s][:, dc, j * 128:(j + 1) * 128],
                            xts[dc][:],
                            start=(dc == 0), stop=(dc == 7))
                if s == "v":
                    nc.tensor.matmul(ps2[:], onesrow[:], b2v_sb[:],
                                     start=False, stop=True)
                    nt_i = t * 4 + j
                    nc.vector.tensor_tensor(
                        v_sb[:, nt_i, :, 0:64],
                        ps2[:].rearrange("p (h d) -> p h d", h=8),
                        silu[:].rearrange("p (h d) -> p h d", h=8),
                        ALU.mult)
                elif s == "q":
                    nc.vector.scalar_tensor_tensor(
                        qt_sb[:, j, t * 512:(t + 1) * 512],
                        ps2[:], b2q[:, j:j + 1], silu[:],
                        op0=ALU.add, op1=ALU.mult)
                else:
                    # K: split into per-parity zero-padded tensors
                    for par in range(2):
                        bp = par * 64
                        nc.vector.scalar_tensor_tensor(
                            ktz[par][bp:bp + 64, j, t * 512:(t + 1) * 512],
                            ps2[bp:bp + 64, :], b2k[bp:bp + 64, j:j + 1],
                            silu[bp:bp + 64, :],
                            op0=ALU.add, op1=ALU.mult)

            A_SUB_COST = {"v": 3900, "k": 3700, "q": 3600}

            def a_unit_items(s, t):
                items = []
                for jh in range(2):
                    for jj in range(2):
                        items.append((A_SUB_COST[s],
                                      lambda s=s, t=t, jh=jh, jj=jj:
                                      a_subunit(s, t, jh, jj)))
                return items

            # ---------------- phase B unit emission ----------------
            mtiles = {}

            def prefetch_mask(qg):
                if mask_mode != "general" or qg in mtiles:
                    return
                mt_sb = mpool.tile([128, NT, 512], MMD, tag="mt")
                qsl = slice(qg * 512, (qg + 1) * 512)
                for kc in range(kc_count(qg)):
                    nc.sync.dma_start(
                        mt_sb[:, kc, :],
                        m01T_d[kc * 128:(kc + 1) * 128, qsl])
                mtiles[qg] = mt_sb

            ct_tiles = {}

            def ctx_ap(ctx, rows, par):
                if isinstance(ctx, tuple):
                    return ctx[par][rows, :]
                return ctx[rows, par, :]

            def b_kc_unit(qg, pj, kc, ctx, kcmax):
                qsl = slice(qg * 512, (qg + 1) * 512)
                ksl = slice(kc * 128, (kc + 1) * 128)
                sc = scps.tile([128, 2, 512], F32, tag="sc", name="sc")
                for par in range(2):
                    nc.tensor.matmul(
                        sc[:, par, :],
                        ktz[par][:, pj, ksl],
                        qt_sb[:, pj, qsl])
                attn = apool.tile([128, 2, 512], MMD, tag="at")
                nc.scalar.activation(attn[:], sc[:], ACTF.Exp)
                if DEBUG and (qg, pj, kc) == (0, 0, 0):
                    nc.sync.dma_start(dbg["attn"][:], attn[:])
                if mask_mode == "causal" and kc >= 4 * qg:
                    nc.vector.tensor_tensor(
                        attn[:], attn[:],
                        pat_sb[:, kc - 4 * qg, None, :].to_broadcast(
                            [128, 2, 512]),
                        ALU.mult)
                elif mask_mode == "general":
                    nc.vector.tensor_tensor(
                        attn[:], attn[:],
                        mtiles[qg][:, kc, None, :].to_broadcast(
                            [128, 2, 512]),
                        ALU.mult)
                for par in range(2):
                    hl = 2 * pj + par
                    nc.tensor.matmul(
                        ctx_ap(ctx, slice(0, 65), par),
                        v_sb[:, kc, hl, :],
                        attn[:, par, :],
                        start=(kc == 0),
                        stop=(kc == kcmax - 1))

            def b_evict_unit(qg, pj, ctx, cxs_box):
                # short critical chain: pull denominators + context out of
                # PSUM so the single ctx buffer frees for the next pj.
                den = den_pp[pj % 2]
                for par in range(2):
                    nc.vector.tensor_copy(den[32 * par:32 * par + 1, :],
                                          ctx_ap(ctx, slice(64, 65), par))
                cxs = cxspool.tile([128, 512], MMD, tag="cxs",
                                   name=f"cxs{qg}_{pj}")
                for par in range(2):
                    nc.vector.tensor_copy(cxs[64 * par:64 * par + 64, :],
                                          ctx_ap(ctx, slice(0, 64), par))
                cxs_box[0] = cxs

            def b_norm_unit(qg, pj, cxs_box):
                # off the critical path: reciprocal, partition-broadcast via
                # bcW matmul, and the normalizing multiply from SBUF staging.
                den = den_pp[pj % 2]
                if mask_mode == "general":
                    nc.vector.reciprocal(rec_f32[:], den[:])
                else:
                    nc.vector.reciprocal_approx_fast(rec_f32[:], den[:])
                nc.vector.tensor_copy(rec_sb[0:33, :], rec_f32[:])
                if DEBUG and (qg, pj) == (0, 0):
                    nc.sync.dma_start(dbg["rec"][:], rec_sb[:])
                bc = scps.tile([128, 2, 512], F32, tag="sc", name="bc")
                nc.tensor.matmul(bc[:, 0, :], bcW[:], rec_sb[:])
                bc_sb = smalls.tile([128, 512], F32, tag="bcs")
                nc.vector.tensor_copy(bc_sb[:], bc[:, 0, :])
                if DEBUG and (qg, pj) == (0, 0):
                    nc.sync.dma_start(dbg["bcw"][:], bcW[:])
                    bcs16 = smalls.tile([128, 512], MMD, tag="bcs16")
                    nc.vector.tensor_copy(bcs16[:], bc_sb[:])
                    nc.sync.dma_start(dbg["bcsb"][:], bcs16[:])
                ct_qg = ct_tiles[qg]
                nc.vector.tensor_tensor(
                    ct_qg[:, pj, :], cxs_box[0][:], bc_sb[:], ALU.mult)

            def b_out_unit(qg, ns, oh):
                nt_i = qg * 4 + ns
                nsl = slice(ns * 128, (ns + 1) * 128)
                ct_qg = ct_tiles[qg]
                po = scps.tile([128, 2, 512], F32, tag="sc", name="po")
                for j in range(4):
                    nc.tensor.matmul(
                        po[:, 0, :],
                        ct_qg[:, j, nsl],
                        woT_sb[:, j, oh * 512:(oh + 1) * 512],
                        start=(j == 0), stop=(j == 3))
                ot = ostage.tile([128, 512], F32, tag="ot")
                nc.vector.tensor_copy(ot[:], po[:, 0, :])
                nc.sync.dma_start(
                    pout_d[nt_i * 128:(nt_i + 1) * 128,
                           oh * 512:(oh + 1) * 512],
                    ot[:])

            def b_qg_items(qg):
                prefetch_mask(qg)
                kcmax = kc_count(qg)
                items = []

                def start_qg(qg=qg):
                    ct_tiles[qg] = ctpool.tile([128, 4, 512], MMD, tag="ct",
                                               name=f"ct{qg}")
                items.append((0, start_qg))
                for pj in range(4):
                    ctx_box = {}

                    def start_pj(ctx_box=ctx_box, qg=qg, pj=pj):
                        if qg == 3 and pj % 2 == 1:
                            # A-phase PSUM banks are free by now; use them
                            # as a second ctx buffer to pipeline pj's.
                            ca = pps.tile([128, 512], F32, tag="pp",
                                          name=f"cxa{qg}_{pj}")
                            cb = pps.tile([128, 512], F32, tag="pp",
                                          name=f"cxb{qg}_{pj}")
                            ctx_box[0] = (ca, cb)
                        else:
                            ctx_box[0] = cxps.tile([128, 2, 512], F32,
                                                   tag="cx",
                                                   name=f"cx{qg}_{pj}")
                    items.append((0, start_pj))
                    for kc in range(kcmax):
                        items.append((900,
                                      lambda qg=qg, pj=pj, kc=kc,
                                      ctx_box=ctx_box, kcmax=kcmax:
                                      b_kc_unit(qg, pj, kc, ctx_box[0],
                                                kcmax)))
                    cxs_box = {}
                    items.append((100,
                                  lambda qg=qg, pj=pj, ctx_box=ctx_box,
                                  cxs_box=cxs_box:
                                  b_evict_unit(qg, pj, ctx_box[0], cxs_box)))
                    items.append((300,
                                  lambda qg=qg, pj=pj, cxs_box=cxs_box:
                                  b_norm_unit(qg, pj, cxs_box)))
                if DEBUG:
                    def dump_ct(qg=qg):
                        nc.sync.dma_start(dbg["ct"][:, qg, :, :],
                                          ct_tiles[qg][:])
                    items.append((0, dump_ct))
                return items

            def b_out_items(qg):
                items = []
                for ns in range(4):
                    for oh in range(2):
                        items.append((900,
                                      lambda qg=qg, ns=ns, oh=oh:
                                      b_out_unit(qg, ns, oh)))
                return items

            # ---------------- interleaved schedule ----------------
            def interleave(a_items, b_items):
                ta = sum(c for c, _ in a_items) or 1
                tb = sum(c for c, _ in b_items) or 1
                ca = cb = 0
                ia = ib = 0
                while ia < len(a_items) or ib < len(b_items):
                    if ib >= len(b_items) or (
                            ia < len(a_items) and ca * tb <= cb * ta):
                        c, f = a_items[ia]
                        ia += 1
                        ca += c
                    else:
                        c, f = b_items[ib]
                        ib += 1
                        cb += c
                    f()

            regions = [
                ([("v", 0), ("k", 0), ("q", 0)], []),
                ([("v", 1), ("k", 1), ("q", 1)], [0]),
                ([("v", 2), ("k", 2), ("q", 2)], [1]),
                ([("v", 3), ("k", 3), ("q", 3)], [2]),
                ([], [3]),
            ]
            # prefetch x for the first region up front
            for s, t in regions[0][0]:
                prefetch_x(s, t)
            for ri, (aunits, bqgs) in enumerate(regions):
                # prefetch next region's x tiles (overlaps this region)
                if ri + 1 < len(regions):
                    for s, t in regions[ri + 1][0]:
                        prefetch_x(s, t)
                a_items = [it for (s, t) in aunits for it in a_unit_items(s, t)]
                b_items = []
                for qg in bqgs:
                    b_items += b_qg_items(qg) + b_out_items(qg)
                interleave(a_items, b_items)
            if DEBUG:
                nc.sync.dma_start(dbg["qt"][:], qt_sb[:])
                nc.sync.dma_start(dbg["kt0"][:], ktz[0][:])
                nc.sync.dma_start(dbg["kt1"][:], ktz[1][:])
                nc.sync.dma_start(dbg["v"][:], v_sb[:])
    nc.compile()
    return nc


def _host_prepare(inputs):
    """Split the full problem into 8 per-core input maps + host-side info."""
    q = np.asarray(inputs["query"], dtype=np.float32)
    k = np.asarray(inputs["key"], dtype=np.float32)
    v = np.asarray(inputs["value"], dtype=np.float32)
    mask = np.asarray(inputs["mask"])
    w = {n: np.asarray(inputs[n], dtype=np.float32)
         for n in ("wq1", "wq2", "wk1", "wk2", "wv1", "wv2", "wo")}
    bias = {n: np.asarray(inputs[n], dtype=np.float32)
            for n in ("bq1", "bq2", "bk1", "bk2", "bv1", "bv2", "bo")}

    m = mask.reshape(S, S)
    if np.array_equal(m != 0, np.tril(np.ones((S, S), bool))):
        mask_mode = "causal"
    elif np.all(m != 0):
        mask_mode = "full"
    else:
        mask_mode = "general"

    pat = None
    m01T = None
    if mask_mode == "causal":
        kk = np.arange(128)[:, None]
        qq = np.arange(512)[None, :]
        pat = np.stack(
            [(kk + 128 * i <= qq).astype(np.float32) for i in range(4)], axis=1
        )  # [128, 4, 512]
        pat = np.ascontiguousarray(pat)
    elif mask_mode == "general":
        m01T = np.ascontiguousarray((m != 0).T.astype(np.float32))

    scale = 1.0 / np.sqrt(DK).astype(np.float32)

    if MM_DTYPE == "bf16":
        import ml_dtypes

        mmd_np = ml_dtypes.bfloat16
    else:
        mmd_np = np.float32

    def cvt(a):
        return np.ascontiguousarray(a).astype(mmd_np)

    in_maps = []
    for c in range(NCORES):
        b, g = divmod(c, 2)
        sl = slice(g * GCH, (g + 1) * GCH)
        im = {
            "xqT": cvt(q[b].T),
            "xkT": cvt(k[b].T),
            "xvT": cvt(v[b].T),
            "w1T_q": cvt(w["wq1"][sl].T),
            # fold the 1/sqrt(dk) score scale into the non-silu Q branch,
            # and 0.5 everywhere (silu computed as A*(1+tanh(A/2)) = 2*silu)
            "w2T_q": cvt(w["wq2"][sl].T * (scale * 0.5)),
            "w2T_k": cvt(w["wk2"][sl].T * 0.5),
            "w2T_v": cvt(w["wv2"][sl].T * 0.5),
            "w1T_k": cvt(w["wk1"][sl].T),
            "w1T_v": cvt(w["wv1"][sl].T),
            "b1_q": np.ascontiguousarray(bias["bq1"][sl].reshape(4, 128).T),
            "b1h_q": np.ascontiguousarray(
                (bias["bq1"][sl] * 0.5).reshape(4, 128).T),
            "b2_q": np.ascontiguousarray(
                (bias["bq2"][sl] * (scale * 0.5)).reshape(4, 128).T),
            "b1_k": np.ascontiguousarray(bias["bk1"][sl].reshape(4, 128).T),
            "b1h_k": np.ascontiguousarray(
                (bias["bk1"][sl] * 0.5).reshape(4, 128).T),
            "b2_k": np.ascontiguousarray(
                (bias["bk2"][sl] * 0.5).reshape(4, 128).T),
            "b1_v": cvt(bias["bv1"][sl].reshape(1, GCH)),
            "b2_v": cvt((bias["bv2"][sl] * 0.5).reshape(1, GCH)),
            "woT": cvt(
                w["wo"][:, sl].T.reshape(4, 128, D).transpose(1, 0, 2)),
        }
        if mask_mode == "causal":
            im["pat"] = cvt(pat)
        elif mask_mode == "general":
            im["m01T"] = cvt(m01T)
        in_maps.append(im)
    return mask_mode, in_maps, bias["bo"]


def kernel(**inputs):
    global LAST_RESULT
    mask_mode, in_maps, bo = _host_prepare(inputs)
    nc = build_program(mask_mode)

    import concourse.bass_utils as bu

    if TRACE:
        import types

        try:
            from trn_agent_boot.trn_boot import _ntff_profile_via_ctypes

            hook = _ntff_profile_via_ctypes("/opt/axon/libaxon_pjrt.so")
            m = types.ModuleType("antenv.axon_hooks")
            m.get_axon_ntff_profile_hook = lambda: hook
            import antenv  # noqa: F401

            sys.modules["antenv.axon_hooks"] = m
            bu.upload_artifacts = lambda d: "local://skipped"
        except Exception as e:
            print("profiling hook install failed:", e)

    res = bu.run_bass_kernel_spmd(
        nc, in_maps, core_ids=list(range(NCORES)),
        trace=TRACE, trace_cores=TRACE_CORES,
    )
    LAST_RESULT = res

    out = np.empty((B, S, D), dtype=np.float32)
    for b in range(B):
        out[b] = (res.results[2 * b]["pout"] + res.results[2 * b + 1]["pout"]
                  + bo[None, :])
    return out


# revision 40
# speedup vs baseline: 1.0529x; 1.0529x over previous
"""SwiGLU-projected causal MHA (B=4, S=2048, D=1024, H=16) on 8 TRN2 NeuronCores.

Sharding: core c -> (batch b = c//2, head-group g = c%2).  Each core computes
the SwiGLU Q/K/V projections for its 512 output channels (= 8 heads) of its
batch, runs causal attention for those heads, and produces a partial output
projection (contraction over its 512 channels).  The host sums the two
partials per batch and adds the output bias.

Key structure (v2):
  - Every matmul is 128x128 mode (K padded to 128 via zeroed weight halves,
    bias matmuls padded with zero rows) -> zero PE mode-switch drains.
  - K tensor stored per-head-parity zero-padded (ktz_e/ktz_o) so score
    matmuls contract K=128 with the other head's rows zeroed.
  - exp batched over [128, 2, 512] PSUM pairs (both heads of a pair per kc).
  - softmax denominator comes out of the AV matmul (ones column in V);
    reciprocal via DVE reciprocal_approx_fast; broadcast across partitions
    via one K=128 matmul with a constant indicator matrix.
  - Phase A (projections) and phase B (attention) are interleaved in
    emission order, paced by estimated PE cost, so exp/DVE work hides under
    projection matmuls and the PE never idles:
      region0: A(v0,k0,q0)
      region1: A(v1,k1,q1) || B(qg0)
      region2: A(v2,k2,q2) || B(qg1)
      region3: A(v3,k3,q3) || B(qg2)   (qg2 only needs t<=2 K/V)
      tail:    B(qg3)
  - PSUM: A pool 2 banks, scores 2x[128,2,512]=4 banks, ctx [128,2,512]=2.
    bc/out-proj tiles borrow the score pool.  Total exactly 8 banks.
"""
import sys

sys.path.insert(0, "/opt/trn_rl_repo")
import numpy as np

import concourse.bacc as bacc
import concourse.tile as tile
import concourse.mybir as mybir

B, S, D = 4, 2048, 1024
H, DK = 16, 64
NCORES = 8
GCH = 512          # channels per core (8 heads)
NT = S // 128      # 16 seq chunks
F32 = mybir.dt.float32
F32R = mybir.dt.float32r
ACTF = mybir.ActivationFunctionType
ALU = mybir.AluOpType

TRACE = False          # set by test.py for profiling runs
TRACE_CORES = None
LAST_RESULT = None     # BassKernelResults stash for test.py
MM_DTYPE = "bf16"
DEBUG = False          # adds intermediate-dump DRAM outputs


def build_program(mask_mode):
    """mask_mode: 'causal' (tril), 'full' (all ones), 'general' (arbitrary)."""
    MMD = mybir.dt.bfloat16 if MM_DTYPE == "bf16" else F32R
    nc = bacc.Bacc("TRN2", target_bir_lowering=False, debug=False)

    xT = {s: nc.dram_tensor(f"x{s}T", [D, S], MMD, kind="ExternalInput")
          for s in "qkv"}
    w1T = {s: nc.dram_tensor(f"w1T_{s}", [D, GCH], MMD, kind="ExternalInput")
           for s in "qkv"}
    w2T = {s: nc.dram_tensor(f"w2T_{s}", [D, GCH], MMD, kind="ExternalInput")
           for s in "qkv"}
    bias_d = {}
    for s in "qk":
        for bn in ("b1", "b2", "b1h"):
            bias_d[f"{bn}_{s}"] = nc.dram_tensor(f"{bn}_{s}", [128, 4], F32,
                                                 kind="ExternalInput")
    b1v_d = nc.dram_tensor("b1_v", [1, GCH], MMD, kind="ExternalInput")
    b2v_d = nc.dram_tensor("b2_v", [1, GCH], MMD, kind="ExternalInput")
    woT_d = nc.dram_tensor("woT", [128, 4, D], MMD, kind="ExternalInput")
    pat_d = m01T_d = None
    if mask_mode == "causal":
        pat_d = nc.dram_tensor("pat", [128, 4, 512], MMD, kind="ExternalInput")
    elif mask_mode == "general":
        m01T_d = nc.dram_tensor("m01T", [S, S], MMD, kind="ExternalInput")
    pout_d = nc.dram_tensor("pout", [S, D], F32, kind="ExternalOutput")
    dbg = {}
    if DEBUG:
        for nm, shp in (("qt", [128, 4, S]), ("kt0", [128, 4, S]),
                        ("kt1", [128, 4, S]), ("v", [128, NT, 8, 65]),
                        ("attn", [128, 2, 512]), ("ct", [128, 4, 4, 512]),
                        ("rec", [128, 512]), ("bcw", [128, 128]),
                        ("bcsb", [128, 512])):
            dbg[nm] = nc.dram_tensor(f"dbg_{nm}", shp, MMD,
                                     kind="ExternalOutput")

    def kc_count(qg):
        return 4 * qg + 4 if mask_mode == "causal" else NT

    with tile.TileContext(nc) as tc:
        with (
            tc.tile_pool(name="persist", bufs=1) as persist,
            tc.tile_pool(name="xpool", bufs=26) as xpool,
            tc.tile_pool(name="stage", bufs=2) as stage,
            tc.tile_pool(name="apool", bufs=4) as apool,
            tc.tile_pool(name="ctpool", bufs=2) as ctpool,
            tc.tile_pool(name="cxspool", bufs=3) as cxspool,
            tc.tile_pool(name="smalls", bufs=2) as smalls,
            tc.tile_pool(name="ostage", bufs=3) as ostage,
            tc.tile_pool(name="mpool", bufs=2) as mpool,
            tc.tile_pool(name="pps", bufs=2, space="PSUM") as pps,
            tc.tile_pool(name="scps", bufs=2, space="PSUM") as scps,
            tc.tile_pool(name="cxps", bufs=1, space="PSUM") as cxps,
        ):
            # ---------------- persistent tiles + preloads ----------------
            qt_sb = persist.tile([128, 4, S], MMD, tag="qt")
            # zero-padded per-parity K: even heads live in partitions 0-63,
            # odd heads in 64-127; the other half stays zero so score
            # matmuls can contract K=128 (no PE mode switch).
            ktz = [persist.tile([128, 4, S], MMD, tag=f"ktz{p}",
                                name=f"ktz{p}")
                   for p in range(2)]
            v_sb = persist.tile([128, NT, 8, 65], MMD, tag="v")
            w1sb = {}
            w2sb = {}
            for s in "vkq":
                w1sb[s] = persist.tile([128, 8, GCH], MMD, tag=f"w1{s}",
                                       name=f"w1{s}")
                w2sb[s] = persist.tile([128, 8, GCH], MMD, tag=f"w2{s}",
                                       name=f"w2{s}")
            woT_sb = persist.tile([128, 4, D], MMD, tag="wo")

            # V biases padded to K=128 (row 0 = bias, rest zero), plus a
            # row-0-ones lhsT so the bias matmul is a full 128x128 matmul.
            b1v_sb = persist.tile([128, GCH], MMD, tag="b1v")
            b2v_sb = persist.tile([128, GCH], MMD, tag="b2v")
            onesrow = persist.tile([128, 128], MMD, tag="onesrow")
            # rec broadcast: bcW[0, 0:64] = 1, bcW[32, 64:128] = 1, rest 0.
            # bc = bcW.T @ rec_sb replicates rec rows 0/32 across partitions.
            bcW = persist.tile([128, 128], MMD, tag="bcW")
            rec_sb = persist.tile([128, 512], MMD, tag="rec")
            rec_f32 = persist.tile([33, 512], F32, tag="recf")
            den_pp = [persist.tile([33, 512], F32, tag=f"den{i}",
                                   name=f"den{i}")
                      for i in range(2)]
            nc.gpsimd.memset(den_pp[0][:], 1.0)
            nc.gpsimd.memset(den_pp[1][:], 1.0)

            nc.gpsimd.memset(ktz[0][:], 0.0)
            nc.gpsimd.memset(ktz[1][:], 0.0)
            nc.gpsimd.memset(b1v_sb[:], 0.0)
            nc.gpsimd.memset(b2v_sb[:], 0.0)
            nc.gpsimd.memset(onesrow[:], 0.0)
            nc.gpsimd.memset(onesrow[0:1, :], 1.0)
            nc.gpsimd.memset(bcW[:], 0.0)
            nc.gpsimd.memset(bcW[0:1, 0:64], 1.0)
            nc.gpsimd.memset(bcW[32:33, 64:128], 1.0)
            nc.gpsimd.memset(rec_sb[:], 0.0)
            nc.gpsimd.memset(rec_f32[:], 0.0)

            onescol = persist.tile([128, 1], F32, tag="onescol")
            nc.any.memset(onescol[:], 1.0)
            nc.vector.tensor_copy(
                v_sb[:, :, :, 64:65],
                onescol[:, None, :].to_broadcast([128, NT, 8, 1]),
            )

            # weight DMAs, v first (first A unit), per-dc chunks so the
            # first matmul can start after ~1.5us of DMA.
            nc.sync.dma_start(b1v_sb[0:1, :], b1v_d[:])
            nc.sync.dma_start(b2v_sb[0:1, :], b2v_d[:])
            for s in "vkq":
                for dc in range(8):
                    nc.sync.dma_start(
                        w1sb[s][:, dc, :],
                        w1T[s][dc * 128:(dc + 1) * 128, :])
                for dc in range(8):
                    nc.sync.dma_start(
                        w2sb[s][:, dc, :],
                        w2T[s][dc * 128:(dc + 1) * 128, :])
                if s != "v":
                    for bn in ("b1", "b2", "b1h"):
                        t_ = persist.tile([128, 4], F32, tag=f"{bn}{s}",
                                          name=f"{bn}{s}")
                        nc.sync.dma_start(t_[:], bias_d[f"{bn}_{s}"][:])
                        if bn == "b1":
                            b1sb = t_
                        elif bn == "b2":
                            b2sb = t_
                        else:
                            b1hsb = t_
                    if s == "k":
                        b1k, b2k, b1hk = b1sb, b2sb, b1hsb
                    else:
                        b1q, b2q, b1hq = b1sb, b2sb, b1hsb
            pat_sb = None
            if mask_mode == "causal":
                pat_sb = persist.tile([128, 4, 512], MMD, tag="pat")
                nc.sync.dma_start(pat_sb[:], pat_d[:])
            nc.sync.dma_start(woT_sb[:], woT_d[:])

            bias_qk = {"q": (b1q, b2q, b1hq), "k": (b1k, b2k, b1hk)}

            # ---------------- phase A unit emission ----------------
            xtiles = {}

            def prefetch_x(s, t):
                if (s, t) in xtiles:
                    return
                xs = []
                for dc in range(8):
                    xt = xpool.tile([128, 512], MMD, tag="xt")
                    nc.sync.dma_start(
                        xt[:],
                        xT[s][dc * 128:(dc + 1) * 128,
                              t * 512:(t + 1) * 512])
                    xs.append(xt)
                xtiles[(s, t)] = xs

            def a_subunit(s, t, jh, jj):
                """One (128 out-ch x 512 seq) SwiGLU tile: ps1 branch fully,
                stash 2*silu, then ps2 branch and write the destination."""
                j = jh * 2 + jj
                xts = xtiles[(s, t)]
                ps1 = pps.tile([128, 512], F32, tag="pp", name="ps1")
                for dc in range(8):
                    if s == "v":
                        nc.tensor.matmul(
                            ps1[:], xts[dc][:, j * 128:(j + 1) * 128],
                            w1sb[s][:, dc, :],
                            start=(dc == 0), stop=False)
                    else:
                        nc.tensor.matmul(
                            ps1[:], w1sb[s][:, dc, j * 128:(j + 1) * 128],
                            xts[dc][:],
                            start=(dc == 0), stop=(dc == 7))
                if s == "v":
                    nc.tensor.matmul(ps1[:], onesrow[:], b1v_sb[:],
                                     start=False, stop=True)
                act = stage.tile([128, 512], F32, tag="act")
                silu = stage.tile([128, 512], F32, tag="silu")
                if s == "v":
                    # act = tanh(ps1/2); silu2 = ps1*(1+act) = 2*silu(ps1)
                    nc.scalar.activation(act[:], ps1[:], ACTF.Tanh, scale=0.5)
                    nc.vector.scalar_tensor_tensor(
                        silu[:], act[:], 1.0, ps1[:],
                        op0=ALU.add, op1=ALU.mult)
                else:
                    b1_, b2_, b1h_ = bias_qk[s]
                    nc.scalar.activation(
                        act[:], ps1[:], ACTF.Tanh,
                        scale=0.5, bias=b1h_[:, j:j + 1])
                    a_sb = stage.tile([128, 512], F32, tag="asb")
                    nc.vector.tensor_scalar_add(a_sb[:], ps1[:],
                                                b1_[:, j:j + 1])
                    nc.vector.scalar_tensor_tensor(
                        silu[:], act[:], 1.0, a_sb[:],
                        op0=ALU.add, op1=ALU.mult)
                ps2 = pps.tile([128, 512], F32, tag="pp", name="ps2")
                for dc in range(8):
                    if s == "v":
                        nc.tensor.matmul(
                            ps2[:], xts[dc][:, j * 128:(j + 1) * 128],
                            w2sb[s][:, dc, :],
                            start=(dc == 0), stop=False)
                    else:
                        nc.tensor.matmul(
                            ps2[:], w2sb[s][:, dc, j * 128:(j + 1) * 128],
                            xts[dc][:],
                            start=(dc == 0), stop=(dc == 7))
                if s == "v":
                    nc.tensor.matmul(ps2[:], onesrow[:], b2v_sb[:],
                                     start=False, stop=True)
                    nt_i = t * 4 + j
                    nc.vector.tensor_tensor(
                        v_sb[:, nt_i, :, 0:64],
                        ps2[:].rearrange("p (h d) -> p h d", h=8),
                        silu[:].rearrange("p (h d) -> p h d", h=8),
                        ALU.mult)
                elif s == "q":
                    nc.vector.scalar_tensor_tensor(
                        qt_sb[:, j, t * 512:(t + 1) * 512],
                        ps2[:], b2q[:, j:j + 1], silu[:],
                        op0=ALU.add, op1=ALU.mult)
                else:
                    # K: split into per-parity zero-padded tensors
                    for par in range(2):
                        bp = par * 64
                        nc.vector.scalar_tensor_tensor(
                            ktz[par][bp:bp + 64, j, t * 512:(t + 1) * 512],
                            ps2[bp:bp + 64, :], b2k[bp:bp + 64, j:j + 1],
                            silu[bp:bp + 64, :],
                            op0=ALU.add, op1=ALU.mult)

            A_SUB_COST = {"v": 3900, "k": 3700, "q": 3600}

            def a_unit_items(s, t):
                items = []
                for jh in range(2):
                    for jj in range(2):
                        items.append((A_SUB_COST[s],
                                      lambda s=s, t=t, jh=jh, jj=jj:
                                      a_subunit(s, t, jh, jj)))
                return items

            # ---------------- phase B unit emission ----------------
            mtiles = {}

            def prefetch_mask(qg):
                if mask_mode != "general" or qg in mtiles:
                    return
                mt_sb = mpool.tile([128, NT, 512], MMD, tag="mt")
                qsl = slice(qg * 512, (qg + 1) * 512)
                for kc in range(kc_count(qg)):
                    nc.sync.dma_start(
                        mt_sb[:, kc, :],
                        m01T_d[kc * 128:(kc + 1) * 128, qsl])
                mtiles[qg] = mt_sb

            ct_tiles = {}

            def ctx_ap(ctx, rows, par):
                if isinstance(ctx, tuple):
                    return ctx[par][rows, :]
                return ctx[rows, par, :]

            def b_kc_unit(qg, pj, kc, ctx, kcmax):
                qsl = slice(qg * 512, (qg + 1) * 512)
                ksl = slice(kc * 128, (kc + 1) * 128)
                sc = scps.tile([128, 2, 512], F32, tag="sc", name="sc")
                for par in range(2):
                    nc.tensor.matmul(
                        sc[:, par, :],
                        ktz[par][:, pj, ksl],
                        qt_sb[:, pj, qsl])
                attn = apool.tile([128, 2, 512], MMD, tag="at")
                nc.scalar.activation(attn[:], sc[:], ACTF.Exp)
                if DEBUG and (qg, pj, kc) == (0, 0, 0):
                    nc.sync.dma_start(dbg["attn"][:], attn[:])
                if mask_mode == "causal" and kc >= 4 * qg:
                    nc.vector.tensor_tensor(
                        attn[:], attn[:],
                        pat_sb[:, kc - 4 * qg, None, :].to_broadcast(
                            [128, 2, 512]),
                        ALU.mult)
                elif mask_mode == "general":
                    nc.vector.tensor_tensor(
                        attn[:], attn[:],
                        mtiles[qg][:, kc, None, :].to_broadcast(
                            [128, 2, 512]),
                        ALU.mult)
                for par in range(2):
                    hl = 2 * pj + par
                    nc.tensor.matmul(
                        ctx_ap(ctx, slice(0, 65), par),
                        v_sb[:, kc, hl, :],
                        attn[:, par, :],
                        start=(kc == 0),
                        stop=(kc == kcmax - 1))

            def b_evict_unit(qg, pj, ctx, cxs_box):
                # short critical chain: pull denominators + context out of
                # PSUM so the single ctx buffer frees for the next pj.
                den = den_pp[pj % 2]
                for par in range(2):
                    nc.vector.tensor_copy(den[32 * par:32 * par + 1, :],
                                          ctx_ap(ctx, slice(64, 65), par))
                cxs = cxspool.tile([128, 512], MMD, tag="cxs",
                                   name=f"cxs{qg}_{pj}")
                for par in range(2):
                    nc.vector.tensor_copy(cxs[64 * par:64 * par + 64, :],
                                          ctx_ap(ctx, slice(0, 64), par))
                cxs_box[0] = cxs

            def b_norm_unit(qg, pj, cxs_box):
                # off the critical path: reciprocal, partition-broadcast via
                # bcW matmul, and the normalizing multiply from SBUF staging.
                den = den_pp[pj % 2]
                if mask_mode == "general":
                    nc.vector.reciprocal(rec_f32[:], den[:])
                else:
                    nc.vector.reciprocal_approx_fast(rec_f32[:], den[:])
                nc.vector.tensor_copy(rec_sb[0:33, :], rec_f32[:])
                if DEBUG and (qg, pj) == (0, 0):
                    nc.sync.dma_start(dbg["rec"][:], rec_sb[:])
                bc = scps.tile([128, 2, 512], F32, tag="sc", name="bc")
                nc.tensor.matmul(bc[:, 0, :], bcW[:], rec_sb[:])
                bc_sb = smalls.tile([128, 512], F32, tag="bcs")
                nc.vector.tensor_copy(bc_sb[:], bc[:, 0, :])
                if DEBUG and (qg, pj) == (0, 0):
                    nc.sync.dma_start(dbg["bcw"][:], bcW[:])
                    bcs16 = smalls.tile([128, 512], MMD, tag="bcs16")
                    nc.vector.tensor_copy(bcs16[:], bc_sb[:])
                    nc.sync.dma_start(dbg["bcsb"][:], bcs16[:])
                ct_qg = ct_tiles[qg]
                nc.vector.tensor_tensor(
                    ct_qg[:, pj, :], cxs_box[0][:], bc_sb[:], ALU.mult)

            def b_out_unit(qg, ns, oh):
                nt_i = qg * 4 + ns
                nsl = slice(ns * 128, (ns + 1) * 128)
                ct_qg = ct_tiles[qg]
                po = scps.tile([128, 2, 512], F32, tag="sc", name="po")
                for j in range(4):
                    nc.tensor.matmul(
                        po[:, 0, :],
                        ct_qg[:, j, nsl],
                        woT_sb[:, j, oh * 512:(oh + 1) * 512],
                        start=(j == 0), stop=(j == 3))
                ot = ostage.tile([128, 512], F32, tag="ot")
                nc.vector.tensor_copy(ot[:], po[:, 0, :])
                nc.sync.dma_start(
                    pout_d[nt_i * 128:(nt_i + 1) * 128,
                           oh * 512:(oh + 1) * 512],
                    ot[:])

            def b_qg_items(qg):
                prefetch_mask(qg)
                kcmax = kc_count(qg)
                items = []

                def start_qg(qg=qg):
                    ct_tiles[qg] = ctpool.tile([128, 4, 512], MMD, tag="ct",
                                               name=f"ct{qg}")
                items.append((0, start_qg))
                pending_norm = None
                for pj in range(4):
                    ctx_box = {}

                    def start_pj(ctx_box=ctx_box, qg=qg, pj=pj):
                        if qg == 3 and pj % 2 == 1:
                            # A-phase PSUM banks are free by now; use them
                            # as a second ctx buffer to pipeline pj's.
                            ca = pps.tile([128, 512], F32, tag="pp",
                                          name=f"cxa{qg}_{pj}")
                            cb = pps.tile([128, 512], F32, tag="pp",
                                          name=f"cxb{qg}_{pj}")
                            ctx_box[0] = (ca, cb)
                        else:
                            ctx_box[0] = cxps.tile([128, 2, 512], F32,
                                                   tag="cx",
                                                   name=f"cx{qg}_{pj}")
                    items.append((0, start_pj))
                    for kc in range(kcmax):
                        items.append((900,
                                      lambda qg=qg, pj=pj, kc=kc,
                                      ctx_box=ctx_box, kcmax=kcmax:
                                      b_kc_unit(qg, pj, kc, ctx_box[0],
                                                kcmax)))
                        # the deferred norm of the previous pj goes a few kc
                        # units in, so its bc matmul never heads the PE FIFO
                        # while its DVE reciprocal chain is still running.
                        if kc == 2 and pending_norm is not None:
                            items.append(pending_norm)
                            pending_norm = None
                    cxs_box = {}
                    items.append((100,
                                  lambda qg=qg, pj=pj, ctx_box=ctx_box,
                                  cxs_box=cxs_box:
                                  b_evict_unit(qg, pj, ctx_box[0], cxs_box)))
                    pending_norm = (300,
                                    lambda qg=qg, pj=pj, cxs_box=cxs_box:
                                    b_norm_unit(qg, pj, cxs_box))
                if pending_norm is not None:
                    items.append(pending_norm)
                    pending_norm = None
                if DEBUG:
                    def dump_ct(qg=qg):
                        nc.sync.dma_start(dbg["ct"][:, qg, :, :],
                                          ct_tiles[qg][:])
                    items.append((0, dump_ct))
                return items

            def b_out_items(qg):
                items = []
                for ns in range(4):
                    for oh in range(2):
                        items.append((900,
                                      lambda qg=qg, ns=ns, oh=oh:
                                      b_out_unit(qg, ns, oh)))
                return items

            # ---------------- interleaved schedule ----------------
            def interleave(a_items, b_items):
                ta = sum(c for c, _ in a_items) or 1
                tb = sum(c for c, _ in b_items) or 1
                ca = cb = 0
                ia = ib = 0
                while ia < len(a_items) or ib < len(b_items):
                    if ib >= len(b_items) or (
                            ia < len(a_items) and ca * tb <= cb * ta):
                        c, f = a_items[ia]
                        ia += 1
                        ca += c
                    else:
                        c, f = b_items[ib]
                        ib += 1
                        cb += c
                    f()

            regions = [
                ([("v", 0), ("k", 0), ("q", 0)], []),
                ([("v", 1), ("k", 1), ("q", 1)], [0]),
                ([("v", 2), ("k", 2), ("q", 2)], [1]),
                ([("v", 3), ("k", 3), ("q", 3)], [2]),
                ([], [3]),
            ]
            # prefetch x for the first region up front
            for s, t in regions[0][0]:
                prefetch_x(s, t)
            for ri, (aunits, bqgs) in enumerate(regions):
                # prefetch next region's x tiles (overlaps this region)
                if ri + 1 < len(regions):
                    for s, t in regions[ri + 1][0]:
                        prefetch_x(s, t)
                a_items = [it for (s, t) in aunits for it in a_unit_items(s, t)]
                b_items = []
                for qg in bqgs:
                    b_items += b_qg_items(qg) + b_out_items(qg)
                interleave(a_items, b_items)
            if DEBUG:
                nc.sync.dma_start(dbg["qt"][:], qt_sb[:])
                nc.sync.dma_start(dbg["kt0"][:], ktz[0][:])
                nc.sync.dma_start(dbg["kt1"][:], ktz[1][:])
                nc.sync.dma_start(dbg["v"][:], v_sb[:])
    nc.compile()
    return nc


def _host_prepare(inputs):
    """Split the full problem into 8 per-core input maps + host-side info."""
    q = np.asarray(inputs["query"], dtype=np.float32)
    k = np.asarray(inputs["key"], dtype=np.float32)
    v = np.asarray(inputs["value"], dtype=np.float32)
    mask = np.asarray(inputs["mask"])
    w = {n: np.asarray(inputs[n], dtype=np.float32)
         for n in ("wq1", "wq2", "wk1", "wk2", "wv1", "wv2", "wo")}
    bias = {n: np.asarray(inputs[n], dtype=np.float32)
            for n in ("bq1", "bq2", "bk1", "bk2", "bv1", "bv2", "bo")}

    m = mask.reshape(S, S)
    if np.array_equal(m != 0, np.tril(np.ones((S, S), bool))):
        mask_mode = "causal"
    elif np.all(m != 0):
        mask_mode = "full"
    else:
        mask_mode = "general"

    pat = None
    m01T = None
    if mask_mode == "causal":
        kk = np.arange(128)[:, None]
        qq = np.arange(512)[None, :]
        pat = np.stack(
            [(kk + 128 * i <= qq).astype(np.float32) for i in range(4)], axis=1
        )  # [128, 4, 512]
        pat = np.ascontiguousarray(pat)
    elif mask_mode == "general":
        m01T = np.ascontiguousarray((m != 0).T.astype(np.float32))

    scale = 1.0 / np.sqrt(DK).astype(np.float32)

    if MM_DTYPE == "bf16":
        import ml_dtypes

        mmd_np = ml_dtypes.bfloat16
    else:
        mmd_np = np.float32

    def cvt(a):
        return np.ascontiguousarray(a).astype(mmd_np)

    in_maps = []
    for c in range(NCORES):
        b, g = divmod(c, 2)
        sl = slice(g * GCH, (g + 1) * GCH)
        im = {
            "xqT": cvt(q[b].T),
            "xkT": cvt(k[b].T),
            "xvT": cvt(v[b].T),
            "w1T_q": cvt(w["wq1"][sl].T),
            # fold the 1/sqrt(dk) score scale into the non-silu Q branch,
            # and 0.5 everywhere (silu computed as A*(1+tanh(A/2)) = 2*silu)
            "w2T_q": cvt(w["wq2"][sl].T * (scale * 0.5)),
            "w2T_k": cvt(w["wk2"][sl].T * 0.5),
            "w2T_v": cvt(w["wv2"][sl].T * 0.5),
            "w1T_k": cvt(w["wk1"][sl].T),
            "w1T_v": cvt(w["wv1"][sl].T),
            "b1_q": np.ascontiguousarray(bias["bq1"][sl].reshape(4, 128).T),
            "b1h_q": np.ascontiguousarray(
                (bias["bq1"][sl] * 0.5).reshape(4, 128).T),
            "b2_q": np.ascontiguousarray(
                (bias["bq2"][sl] * (scale * 0.5)).reshape(4, 128).T),
            "b1_k": np.ascontiguousarray(bias["bk1"][sl].reshape(4, 128).T),
            "b1h_k": np.ascontiguousarray(
                (bias["bk1"][sl] * 0.5).reshape(4, 128).T),
            "b2_k": np.ascontiguousarray(
                (bias["bk2"][sl] * 0.5).reshape(4, 128).T),
            "b1_v": cvt(bias["bv1"][sl].reshape(1, GCH)),
            "b2_v": cvt((bias["bv2"][sl] * 0.5).reshape(1, GCH)),
            "woT": cvt(
                w["wo"][:, sl].T.reshape(4, 128, D).transpose(1, 0, 2)),
        }
        if mask_mode == "causal":
            im["pat"] = cvt(pat)
        elif mask_mode == "general":
            im["m01T"] = cvt(m01T)
        in_maps.append(im)
    return mask_mode, in_maps, bias["bo"]


def kernel(**inputs):
    global LAST_RESULT
    mask_mode, in_maps, bo = _host_prepare(inputs)
    nc = build_program(mask_mode)

    import concourse.bass_utils as bu

    if TRACE:
        import types

        try:
            from trn_agent_boot.trn_boot import _ntff_profile_via_ctypes

            hook = _ntff_profile_via_ctypes("/opt/axon/libaxon_pjrt.so")
            m = types.ModuleType("antenv.axon_hooks")
            m.get_axon_ntff_profile_hook = lambda: hook
            import antenv  # noqa: F401

            sys.modules["antenv.axon_hooks"] = m
            bu.upload_artifacts = lambda d: "local://skipped"
        except Exception as e:
            print("profiling hook install failed:", e)

    res = bu.run_bass_kernel_spmd(
        nc, in_maps, core_ids=list(range(NCORES)),
        trace=TRACE, trace_cores=TRACE_CORES,
    )
    LAST_RESULT = res

    out = np.empty((B, S, D), dtype=np.float32)
    for b in range(B):
        out[b] = (res.results[2 * b]["pout"] + res.results[2 * b + 1]["pout"]
                  + bo[None, :])
    return out


# revision 44
# speedup vs baseline: 1.0961x; 1.0410x over previous
"""SwiGLU-projected causal MHA (B=4, S=2048, D=1024, H=16) on 8 TRN2 NeuronCores.

Sharding: core c -> (batch b = c//2, head-group g = c%2).  Each core computes
the SwiGLU Q/K/V projections for its 512 output channels (= 8 heads) of its
batch, runs causal attention for those heads, and produces a partial output
projection (contraction over its 512 channels).  The host sums the two
partials per batch and adds the output bias.

Key structure (v2):
  - Every matmul is 128x128 mode (K padded to 128 via zeroed weight halves,
    bias matmuls padded with zero rows) -> zero PE mode-switch drains.
  - K tensor stored per-head-parity zero-padded (ktz_e/ktz_o) so score
    matmuls contract K=128 with the other head's rows zeroed.
  - exp batched over [128, 2, 512] PSUM pairs (both heads of a pair per kc).
  - softmax denominator comes out of the AV matmul (ones column in V);
    reciprocal via DVE reciprocal_approx_fast; broadcast across partitions
    via one K=128 matmul with a constant indicator matrix.
  - Phase A (projections) and phase B (attention) are interleaved in
    emission order, paced by estimated PE cost, so exp/DVE work hides under
    projection matmuls and the PE never idles:
      region0: A(v0,k0,q0)
      region1: A(v1,k1,q1) || B(qg0)
      region2: A(v2,k2,q2) || B(qg1)
      region3: A(v3,k3,q3) || B(qg2)   (qg2 only needs t<=2 K/V)
      tail:    B(qg3)
  - PSUM: A pool 2 banks, scores 2x[128,2,512]=4 banks, ctx [128,2,512]=2.
    bc/out-proj tiles borrow the score pool.  Total exactly 8 banks.
"""
import sys

sys.path.insert(0, "/opt/trn_rl_repo")
import numpy as np

import concourse.bacc as bacc
import concourse.tile as tile
import concourse.mybir as mybir

B, S, D = 4, 2048, 1024
H, DK = 16, 64
NCORES = 8
GCH = 512          # channels per core (8 heads)
NT = S // 128      # 16 seq chunks
F32 = mybir.dt.float32
F32R = mybir.dt.float32r
ACTF = mybir.ActivationFunctionType
ALU = mybir.AluOpType

TRACE = False          # set by test.py for profiling runs
TRACE_CORES = None
LAST_RESULT = None     # BassKernelResults stash for test.py
MM_DTYPE = "bf16"
DEBUG = False          # adds intermediate-dump DRAM outputs


def build_program(mask_mode):
    """mask_mode: 'causal' (tril), 'full' (all ones), 'general' (arbitrary)."""
    MMD = mybir.dt.bfloat16 if MM_DTYPE == "bf16" else F32R
    nc = bacc.Bacc("TRN2", target_bir_lowering=False, debug=False)

    xT = {s: nc.dram_tensor(f"x{s}T", [D, S], MMD, kind="ExternalInput")
          for s in "qkv"}
    w1T = {s: nc.dram_tensor(f"w1T_{s}", [D, GCH], MMD, kind="ExternalInput")
           for s in "qkv"}
    w2T = {s: nc.dram_tensor(f"w2T_{s}", [D, GCH], MMD, kind="ExternalInput")
           for s in "qkv"}
    bias_d = {}
    for s in "qk":
        for bn in ("b1", "b2", "b1h"):
            bias_d[f"{bn}_{s}"] = nc.dram_tensor(f"{bn}_{s}", [128, 4], F32,
                                                 kind="ExternalInput")
    b1v_d = nc.dram_tensor("b1_v", [1, GCH], MMD, kind="ExternalInput")
    b2v_d = nc.dram_tensor("b2_v", [1, GCH], MMD, kind="ExternalInput")
    woT_d = nc.dram_tensor("woT", [128, 4, D], MMD, kind="ExternalInput")
    pat_d = m01T_d = None
    if mask_mode == "causal":
        pat_d = nc.dram_tensor("pat", [128, 4, 512], MMD, kind="ExternalInput")
    elif mask_mode == "general":
        m01T_d = nc.dram_tensor("m01T", [S, S], MMD, kind="ExternalInput")
    pout_d = nc.dram_tensor("pout", [S, D], F32, kind="ExternalOutput")
    dbg = {}
    if DEBUG:
        for nm, shp in (("qt", [128, 4, S]), ("kt0", [128, 4, S]),
                        ("kt1", [128, 4, S]), ("v", [128, NT, 8, 65]),
                        ("attn", [128, 2, 512]), ("ct", [128, 4, 4, 512]),
                        ("rec", [128, 512]), ("bcw", [128, 128]),
                        ("bcsb", [128, 512])):
            dbg[nm] = nc.dram_tensor(f"dbg_{nm}", shp, MMD,
                                     kind="ExternalOutput")

    def kc_count(qg):
        return 4 * qg + 4 if mask_mode == "causal" else NT

    with tile.TileContext(nc) as tc:
        with (
            tc.tile_pool(name="persist", bufs=1) as persist,
            tc.tile_pool(name="xpool", bufs=26) as xpool,
            tc.tile_pool(name="stage", bufs=2) as stage,
            tc.tile_pool(name="apool", bufs=4) as apool,
            tc.tile_pool(name="ctpool", bufs=2) as ctpool,
            tc.tile_pool(name="cxspool", bufs=3) as cxspool,
            tc.tile_pool(name="smalls", bufs=2) as smalls,
            tc.tile_pool(name="ostage", bufs=3) as ostage,
            tc.tile_pool(name="mpool", bufs=2) as mpool,
            tc.tile_pool(name="pps", bufs=2, space="PSUM") as pps,
            tc.tile_pool(name="scps", bufs=2, space="PSUM") as scps,
            tc.tile_pool(name="cxps", bufs=1, space="PSUM") as cxps,
        ):
            # ---------------- persistent tiles + preloads ----------------
            qt_sb = persist.tile([128, 4, S], MMD, tag="qt")
            # zero-padded per-parity K: even heads live in partitions 0-63,
            # odd heads in 64-127; the other half stays zero so score
            # matmuls can contract K=128 (no PE mode switch).
            ktz = [persist.tile([128, 4, S], MMD, tag=f"ktz{p}",
                                name=f"ktz{p}")
                   for p in range(2)]
            v_sb = persist.tile([128, NT, 8, 65], MMD, tag="v")
            w1sb = {}
            w2sb = {}
            for s in "vkq":
                w1sb[s] = persist.tile([128, 8, GCH], MMD, tag=f"w1{s}",
                                       name=f"w1{s}")
                w2sb[s] = persist.tile([128, 8, GCH], MMD, tag=f"w2{s}",
                                       name=f"w2{s}")
            woT_sb = persist.tile([128, 4, D], MMD, tag="wo")

            # V biases padded to K=128 (row 0 = bias, rest zero), plus a
            # row-0-ones lhsT so the bias matmul is a full 128x128 matmul.
            b1v_sb = persist.tile([128, GCH], MMD, tag="b1v")
            b2v_sb = persist.tile([128, GCH], MMD, tag="b2v")
            onesrow = persist.tile([128, 128], MMD, tag="onesrow")
            # rec broadcast: bcW[0, 0:64] = 1, bcW[32, 64:128] = 1, rest 0.
            # bc = bcW.T @ rec_sb replicates rec rows 0/32 across partitions.
            bcW = persist.tile([128, 128], MMD, tag="bcW")
            rec_sb = persist.tile([128, 512], MMD, tag="rec")
            rec_f32 = persist.tile([33, 512], F32, tag="recf")
            den_pp = [persist.tile([33, 512], F32, tag=f"den{i}",
                                   name=f"den{i}")
                      for i in range(2)]
            nc.gpsimd.memset(den_pp[0][:], 1.0)
            nc.gpsimd.memset(den_pp[1][:], 1.0)

            nc.gpsimd.memset(ktz[0][:], 0.0)
            nc.gpsimd.memset(ktz[1][:], 0.0)
            nc.gpsimd.memset(b1v_sb[:], 0.0)
            nc.gpsimd.memset(b2v_sb[:], 0.0)
            nc.gpsimd.memset(onesrow[:], 0.0)
            nc.gpsimd.memset(onesrow[0:1, :], 1.0)
            nc.gpsimd.memset(bcW[:], 0.0)
            nc.gpsimd.memset(bcW[0:1, 0:64], 1.0)
            nc.gpsimd.memset(bcW[32:33, 64:128], 1.0)
            nc.gpsimd.memset(rec_sb[:], 0.0)
            nc.gpsimd.memset(rec_f32[:], 0.0)

            onescol = persist.tile([128, 1], F32, tag="onescol")
            nc.any.memset(onescol[:], 1.0)
            nc.vector.tensor_copy(
                v_sb[:, :, :, 64:65],
                onescol[:, None, :].to_broadcast([128, NT, 8, 1]),
            )

            # x-tile prefetch machinery (defined early so weight DMAs can
            # interleave with the first x tiles on the sync queue).
            xtiles = {}

            def prefetch_x(s, t):
                if (s, t) in xtiles:
                    return
                xs = []
                for dc in range(8):
                    xt = xpool.tile([128, 512], MMD, tag="xt")
                    nc.sync.dma_start(
                        xt[:],
                        xT[s][dc * 128:(dc + 1) * 128,
                              t * 512:(t + 1) * 512])
                    xs.append(xt)
                xtiles[(s, t)] = xs

            # weight DMAs interleaved with the x tiles each A unit needs,
            # so the first matmul starts after ~8us instead of ~50us.
            bias_qk = {}
            for s in "vkq":
                if s == "v":
                    nc.sync.dma_start(b1v_sb[0:1, :], b1v_d[:])
                    nc.sync.dma_start(b2v_sb[0:1, :], b2v_d[:])
                for dc in range(8):
                    nc.sync.dma_start(
                        w1sb[s][:, dc, :],
                        w1T[s][dc * 128:(dc + 1) * 128, :])
                for dc in range(8):
                    nc.sync.dma_start(
                        w2sb[s][:, dc, :],
                        w2T[s][dc * 128:(dc + 1) * 128, :])
                if s != "v":
                    bb = []
                    for bn in ("b1", "b2", "b1h"):
                        t_ = persist.tile([128, 4], F32, tag=f"{bn}{s}",
                                          name=f"{bn}{s}")
                        nc.sync.dma_start(t_[:], bias_d[f"{bn}_{s}"][:])
                        bb.append(t_)
                    bias_qk[s] = tuple(bb)
                prefetch_x(s, 0)
            pat_sb = None
            if mask_mode == "causal":
                pat_sb = persist.tile([128, 4, 512], MMD, tag="pat")
                nc.sync.dma_start(pat_sb[:], pat_d[:])
            nc.sync.dma_start(woT_sb[:], woT_d[:])

            # ---------------- phase A unit emission ----------------
            def a_subunit(s, t, jh, jj):
                """One (128 out-ch x 512 seq) SwiGLU tile: ps1 branch fully,
                stash 2*silu, then ps2 branch and write the destination."""
                j = jh * 2 + jj
                xts = xtiles[(s, t)]
                ps1 = pps.tile([128, 512], F32, tag="pp", name="ps1")
                for dc in range(8):
                    if s == "v":
                        nc.tensor.matmul(
                            ps1[:], xts[dc][:, j * 128:(j + 1) * 128],
                            w1sb[s][:, dc, :],
                            start=(dc == 0), stop=False)
                    else:
                        nc.tensor.matmul(
                            ps1[:], w1sb[s][:, dc, j * 128:(j + 1) * 128],
                            xts[dc][:],
                            start=(dc == 0), stop=(dc == 7))
                if s == "v":
                    nc.tensor.matmul(ps1[:], onesrow[:], b1v_sb[:],
                                     start=False, stop=True)
                act = stage.tile([128, 512], F32, tag="act")
                silu = stage.tile([128, 512], F32, tag="silu")
                if s == "v":
                    # act = tanh(ps1/2); silu2 = ps1*(1+act) = 2*silu(ps1)
                    nc.scalar.activation(act[:], ps1[:], ACTF.Tanh, scale=0.5)
                    nc.vector.scalar_tensor_tensor(
                        silu[:], act[:], 1.0, ps1[:],
                        op0=ALU.add, op1=ALU.mult)
                else:
                    b1_, b2_, b1h_ = bias_qk[s]
                    nc.scalar.activation(
                        act[:], ps1[:], ACTF.Tanh,
                        scale=0.5, bias=b1h_[:, j:j + 1])
                    a_sb = stage.tile([128, 512], F32, tag="asb")
                    nc.vector.tensor_scalar_add(a_sb[:], ps1[:],
                                                b1_[:, j:j + 1])
                    nc.vector.scalar_tensor_tensor(
                        silu[:], act[:], 1.0, a_sb[:],
                        op0=ALU.add, op1=ALU.mult)
                ps2 = pps.tile([128, 512], F32, tag="pp", name="ps2")
                for dc in range(8):
                    if s == "v":
                        nc.tensor.matmul(
                            ps2[:], xts[dc][:, j * 128:(j + 1) * 128],
                            w2sb[s][:, dc, :],
                            start=(dc == 0), stop=False)
                    else:
                        nc.tensor.matmul(
                            ps2[:], w2sb[s][:, dc, j * 128:(j + 1) * 128],
                            xts[dc][:],
                            start=(dc == 0), stop=(dc == 7))
                if s == "v":
                    nc.tensor.matmul(ps2[:], onesrow[:], b2v_sb[:],
                                     start=False, stop=True)
                    nt_i = t * 4 + j
                    nc.vector.tensor_tensor(
                        v_sb[:, nt_i, :, 0:64],
                        ps2[:].rearrange("p (h d) -> p h d", h=8),
                        silu[:].rearrange("p (h d) -> p h d", h=8),
                        ALU.mult)
                elif s == "q":
                    nc.vector.scalar_tensor_tensor(
                        qt_sb[:, j, t * 512:(t + 1) * 512],
                        ps2[:], bias_qk["q"][1][:, j:j + 1], silu[:],
                        op0=ALU.add, op1=ALU.mult)
                else:
                    # K: split into per-parity zero-padded tensors
                    for par in range(2):
                        bp = par * 64
                        nc.vector.scalar_tensor_tensor(
                            ktz[par][bp:bp + 64, j, t * 512:(t + 1) * 512],
                            ps2[bp:bp + 64, :],
                            bias_qk["k"][1][bp:bp + 64, j:j + 1],
                            silu[bp:bp + 64, :],
                            op0=ALU.add, op1=ALU.mult)

            A_SUB_COST = {"v": 3900, "k": 3700, "q": 3600}

            def a_unit_items(s, t):
                items = []
                for jh in range(2):
                    for jj in range(2):
                        items.append((A_SUB_COST[s],
                                      lambda s=s, t=t, jh=jh, jj=jj:
                                      a_subunit(s, t, jh, jj)))
                return items

            # ---------------- phase B unit emission ----------------
            mtiles = {}

            def prefetch_mask(qg):
                if mask_mode != "general" or qg in mtiles:
                    return
                mt_sb = mpool.tile([128, NT, 512], MMD, tag="mt")
                qsl = slice(qg * 512, (qg + 1) * 512)
                for kc in range(kc_count(qg)):
                    nc.sync.dma_start(
                        mt_sb[:, kc, :],
                        m01T_d[kc * 128:(kc + 1) * 128, qsl])
                mtiles[qg] = mt_sb

            ct_tiles = {}

            def ctx_ap(ctx, rows, par):
                if isinstance(ctx, tuple):
                    return ctx[par][rows, :]
                return ctx[rows, par, :]

            def b_kc_unit(qg, pj, kc, ctx, kcmax):
                qsl = slice(qg * 512, (qg + 1) * 512)
                ksl = slice(kc * 128, (kc + 1) * 128)
                sc = scps.tile([128, 2, 512], F32, tag="sc", name="sc")
                for par in range(2):
                    nc.tensor.matmul(
                        sc[:, par, :],
                        ktz[par][:, pj, ksl],
                        qt_sb[:, pj, qsl])
                attn = apool.tile([128, 2, 512], MMD, tag="at")
                nc.scalar.activation(attn[:], sc[:], ACTF.Exp)
                if DEBUG and (qg, pj, kc) == (0, 0, 0):
                    nc.sync.dma_start(dbg["attn"][:], attn[:])
                if mask_mode == "causal" and kc >= 4 * qg:
                    nc.vector.tensor_tensor(
                        attn[:], attn[:],
                        pat_sb[:, kc - 4 * qg, None, :].to_broadcast(
                            [128, 2, 512]),
                        ALU.mult)
                elif mask_mode == "general":
                    nc.vector.tensor_tensor(
                        attn[:], attn[:],
                        mtiles[qg][:, kc, None, :].to_broadcast(
                            [128, 2, 512]),
                        ALU.mult)
                for par in range(2):
                    hl = 2 * pj + par
                    nc.tensor.matmul(
                        ctx_ap(ctx, slice(0, 65), par),
                        v_sb[:, kc, hl, :],
                        attn[:, par, :],
                        start=(kc == 0),
                        stop=(kc == kcmax - 1))

            def b_evict_unit(qg, pj, ctx, cxs_box):
                # short critical chain: pull denominators + context out of
                # PSUM so the single ctx buffer frees for the next pj.
                den = den_pp[pj % 2]
                for par in range(2):
                    nc.vector.tensor_copy(den[32 * par:32 * par + 1, :],
                                          ctx_ap(ctx, slice(64, 65), par))
                cxs = cxspool.tile([128, 512], MMD, tag="cxs",
                                   name=f"cxs{qg}_{pj}")
                for par in range(2):
                    nc.vector.tensor_copy(cxs[64 * par:64 * par + 64, :],
                                          ctx_ap(ctx, slice(0, 64), par))
                cxs_box[0] = cxs

            def b_norm_unit(qg, pj, cxs_box):
                # off the critical path: reciprocal, partition-broadcast via
                # bcW matmul, and the normalizing multiply from SBUF staging.
                den = den_pp[pj % 2]
                if mask_mode == "general":
                    nc.vector.reciprocal(rec_f32[:], den[:])
                else:
                    nc.vector.reciprocal_approx_fast(rec_f32[:], den[:])
                nc.vector.tensor_copy(rec_sb[0:33, :], rec_f32[:])
                if DEBUG and (qg, pj) == (0, 0):
                    nc.sync.dma_start(dbg["rec"][:], rec_sb[:])
                bc = scps.tile([128, 2, 512], F32, tag="sc", name="bc")
                nc.tensor.matmul(bc[:, 0, :], bcW[:], rec_sb[:])
                bc_sb = smalls.tile([128, 512], F32, tag="bcs")
                nc.vector.tensor_copy(bc_sb[:], bc[:, 0, :])
                if DEBUG and (qg, pj) == (0, 0):
                    nc.sync.dma_start(dbg["bcw"][:], bcW[:])
                    bcs16 = smalls.tile([128, 512], MMD, tag="bcs16")
                    nc.vector.tensor_copy(bcs16[:], bc_sb[:])
                    nc.sync.dma_start(dbg["bcsb"][:], bcs16[:])
                ct_qg = ct_tiles[qg]
                nc.vector.tensor_tensor(
                    ct_qg[:, pj, :], cxs_box[0][:], bc_sb[:], ALU.mult)

            def b_out_unit(qg, ns, oh):
                nt_i = qg * 4 + ns
                nsl = slice(ns * 128, (ns + 1) * 128)
                ct_qg = ct_tiles[qg]
                po = scps.tile([128, 2, 512], F32, tag="sc", name="po")
                for j in range(4):
                    nc.tensor.matmul(
                        po[:, 0, :],
                        ct_qg[:, j, nsl],
                        woT_sb[:, j, oh * 512:(oh + 1) * 512],
                        start=(j == 0), stop=(j == 3))
                ot = ostage.tile([128, 512], F32, tag="ot")
                nc.vector.tensor_copy(ot[:], po[:, 0, :])
                nc.sync.dma_start(
                    pout_d[nt_i * 128:(nt_i + 1) * 128,
                           oh * 512:(oh + 1) * 512],
                    ot[:])

            def b_qg_items(qg):
                prefetch_mask(qg)
                kcmax = kc_count(qg)
                items = []

                def start_qg(qg=qg):
                    ct_tiles[qg] = ctpool.tile([128, 4, 512], MMD, tag="ct",
                                               name=f"ct{qg}")
                items.append((0, start_qg))
                pending_norm = None
                for pj in range(4):
                    ctx_box = {}

                    def start_pj(ctx_box=ctx_box, qg=qg, pj=pj):
                        if qg == 3 and pj % 2 == 1:
                            # A-phase PSUM banks are free by now; use them
                            # as a second ctx buffer to pipeline pj's.
                            ca = pps.tile([128, 512], F32, tag="pp",
                                          name=f"cxa{qg}_{pj}")
                            cb = pps.tile([128, 512], F32, tag="pp",
                                          name=f"cxb{qg}_{pj}")
                            ctx_box[0] = (ca, cb)
                        else:
                            ctx_box[0] = cxps.tile([128, 2, 512], F32,
                                                   tag="cx",
                                                   name=f"cx{qg}_{pj}")
                    items.append((0, start_pj))
                    for kc in range(kcmax):
                        items.append((900,
                                      lambda qg=qg, pj=pj, kc=kc,
                                      ctx_box=ctx_box, kcmax=kcmax:
                                      b_kc_unit(qg, pj, kc, ctx_box[0],
                                                kcmax)))
                        # the deferred norm of the previous pj goes a few kc
                        # units in, so its bc matmul never heads the PE FIFO
                        # while its DVE reciprocal chain is still running.
                        if kc == 2 and pending_norm is not None:
                            items.append(pending_norm)
                            pending_norm = None
                    cxs_box = {}
                    items.append((100,
                                  lambda qg=qg, pj=pj, ctx_box=ctx_box,
                                  cxs_box=cxs_box:
                                  b_evict_unit(qg, pj, ctx_box[0], cxs_box)))
                    pending_norm = (300,
                                    lambda qg=qg, pj=pj, cxs_box=cxs_box:
                                    b_norm_unit(qg, pj, cxs_box))
                if pending_norm is not None:
                    items.append(pending_norm)
                    pending_norm = None
                if DEBUG:
                    def dump_ct(qg=qg):
                        nc.sync.dma_start(dbg["ct"][:, qg, :, :],
                                          ct_tiles[qg][:])
                    items.append((0, dump_ct))
                return items

            def b_out_items(qg):
                items = []
                for ns in range(4):
                    for oh in range(2):
                        items.append((900,
                                      lambda qg=qg, ns=ns, oh=oh:
                                      b_out_unit(qg, ns, oh)))
                return items

            # ---------------- interleaved schedule ----------------
            def interleave(a_items, b_items):
                ta = sum(c for c, _ in a_items) or 1
                tb = sum(c for c, _ in b_items) or 1
                ca = cb = 0
                ia = ib = 0
                while ia < len(a_items) or ib < len(b_items):
                    if ib >= len(b_items) or (
                            ia < len(a_items) and ca * tb <= cb * ta):
                        c, f = a_items[ia]
                        ia += 1
                        ca += c
                    else:
                        c, f = b_items[ib]
                        ib += 1
                        cb += c
                    f()

            regions = [
                ([("v", 0), ("k", 0), ("q", 0)], []),
                ([("v", 1), ("k", 1), ("q", 1)], [0]),
                ([("v", 2), ("k", 2), ("q", 2)], [1]),
                ([("v", 3), ("k", 3), ("q", 3)], [2]),
                ([], [3]),
            ]
            # prefetch x for the first region up front
            for s, t in regions[0][0]:
                prefetch_x(s, t)
            for ri, (aunits, bqgs) in enumerate(regions):
                # prefetch next region's x tiles (overlaps this region)
                if ri + 1 < len(regions):
                    for s, t in regions[ri + 1][0]:
                        prefetch_x(s, t)
                a_items = [it for (s, t) in aunits for it in a_unit_items(s, t)]
                b_items = []
                for qg in bqgs:
                    b_items += b_qg_items(qg) + b_out_items(qg)
                interleave(a_items, b_items)
            if DEBUG:
                nc.sync.dma_start(dbg["qt"][:], qt_sb[:])
                nc.sync.dma_start(dbg["kt0"][:], ktz[0][:])
                nc.sync.dma_start(dbg["kt1"][:], ktz[1][:])
                nc.sync.dma_start(dbg["v"][:], v_sb[:])
    nc.compile()
    return nc


def _host_prepare(inputs):
    """Split the full problem into 8 per-core input maps + host-side info."""
    q = np.asarray(inputs["query"], dtype=np.float32)
    k = np.asarray(inputs["key"], dtype=np.float32)
    v = np.asarray(inputs["value"], dtype=np.float32)
    mask = np.asarray(inputs["mask"])
    w = {n: np.asarray(inputs[n], dtype=np.float32)
         for n in ("wq1", "wq2", "wk1", "wk2", "wv1", "wv2", "wo")}
    bias = {n: np.asarray(inputs[n], dtype=np.float32)
            for n in ("bq1", "bq2", "bk1", "bk2", "bv1", "bv2", "bo")}

    m = mask.reshape(S, S)
    if np.array_equal(m != 0, np.tril(np.ones((S, S), bool))):
        mask_mode = "causal"
    elif np.all(m != 0):
        mask_mode = "full"
    else:
        mask_mode = "general"

    pat = None
    m01T = None
    if mask_mode == "causal":
        kk = np.arange(128)[:, None]
        qq = np.arange(512)[None, :]
        pat = np.stack(
            [(kk + 128 * i <= qq).astype(np.float32) for i in range(4)], axis=1
        )  # [128, 4, 512]
        pat = np.ascontiguousarray(pat)
    elif mask_mode == "general":
        m01T = np.ascontiguousarray((m != 0).T.astype(np.float32))

    scale = 1.0 / np.sqrt(DK).astype(np.float32)

    if MM_DTYPE == "bf16":
        import ml_dtypes

        mmd_np = ml_dtypes.bfloat16
    else:
        mmd_np = np.float32

    def cvt(a):
        return np.ascontiguousarray(a).astype(mmd_np)

    in_maps = []
    for c in range(NCORES):
        b, g = divmod(c, 2)
        sl = slice(g * GCH, (g + 1) * GCH)
        im = {
            "xqT": cvt(q[b].T),
            "xkT": cvt(k[b].T),
            "xvT": cvt(v[b].T),
            "w1T_q": cvt(w["wq1"][sl].T),
            # fold the 1/sqrt(dk) score scale into the non-silu Q branch,
            # and 0.5 everywhere (silu computed as A*(1+tanh(A/2)) = 2*silu)
            "w2T_q": cvt(w["wq2"][sl].T * (scale * 0.5)),
            "w2T_k": cvt(w["wk2"][sl].T * 0.5),
            "w2T_v": cvt(w["wv2"][sl].T * 0.5),
            "w1T_k": cvt(w["wk1"][sl].T),
            "w1T_v": cvt(w["wv1"][sl].T),
            "b1_q": np.ascontiguousarray(bias["bq1"][sl].reshape(4, 128).T),
            "b1h_q": np.ascontiguousarray(
                (bias["bq1"][sl] * 0.5).reshape(4, 128).T),
            "b2_q": np.ascontiguousarray(
                (bias["bq2"][sl] * (scale * 0.5)).reshape(4, 128).T),
            "b1_k": np.ascontiguousarray(bias["bk1"][sl].reshape(4, 128).T),
            "b1h_k": np.ascontiguousarray(
                (bias["bk1"][sl] * 0.5).reshape(4, 128).T),
            "b2_k": np.ascontiguousarray(
                (bias["bk2"][sl] * 0.5).reshape(4, 128).T),
            "b1_v": cvt(bias["bv1"][sl].reshape(1, GCH)),
            "b2_v": cvt((bias["bv2"][sl] * 0.5).reshape(1, GCH)),
            "woT": cvt(
                w["wo"][:, sl].T.reshape(4, 128, D).transpose(1, 0, 2)),
        }
        if mask_mode == "causal":
            im["pat"] = cvt(pat)
        elif mask_mode == "general":
            im["m01T"] = cvt(m01T)
        in_maps.append(im)
    return mask_mode, in_maps, bias["bo"]


def kernel(**inputs):
    global LAST_RESULT
    mask_mode, in_maps, bo = _host_prepare(inputs)
    nc = build_program(mask_mode)

    import concourse.bass_utils as bu

    if TRACE:
        import types

        try:
            from trn_agent_boot.trn_boot import _ntff_profile_via_ctypes

            hook = _ntff_profile_via_ctypes("/opt/axon/libaxon_pjrt.so")
            m = types.ModuleType("antenv.axon_hooks")
            m.get_axon_ntff_profile_hook = lambda: hook
            import antenv  # noqa: F401

            sys.modules["antenv.axon_hooks"] = m
            bu.upload_artifacts = lambda d: "local://skipped"
        except Exception as e:
            print("profiling hook install failed:", e)

    res = bu.run_bass_kernel_spmd(
        nc, in_maps, core_ids=list(range(NCORES)),
        trace=TRACE, trace_cores=TRACE_CORES,
    )
    LAST_RESULT = res

    out = np.empty((B, S, D), dtype=np.float32)
    for b in range(B):
        out[b] = (res.results[2 * b]["pout"] + res.results[2 * b + 1]["pout"]
                  + bo[None, :])
    return out


# revision 46
# speedup vs baseline: 1.1724x; 1.0696x over previous
"""SwiGLU-projected causal MHA (B=4, S=2048, D=1024, H=16) on 8 TRN2 NeuronCores.

Sharding: core c -> (batch b = c//2, head-group g = c%2).  Each core computes
the SwiGLU Q/K/V projections for its 512 output channels (= 8 heads) of its
batch, runs causal attention for those heads, and produces a partial output
projection (contraction over its 512 channels).  The host sums the two
partials per batch and adds the output bias.

Key structure (v2):
  - Every matmul is 128x128 mode (K padded to 128 via zeroed weight halves,
    bias matmuls padded with zero rows) -> zero PE mode-switch drains.
  - K tensor stored per-head-parity zero-padded (ktz_e/ktz_o) so score
    matmuls contract K=128 with the other head's rows zeroed.
  - exp batched over [128, 2, 512] PSUM pairs (both heads of a pair per kc).
  - softmax denominator comes out of the AV matmul (ones column in V);
    reciprocal via DVE reciprocal_approx_fast; broadcast across partitions
    via one K=128 matmul with a constant indicator matrix.
  - Phase A (projections) and phase B (attention) are interleaved in
    emission order, paced by estimated PE cost, so exp/DVE work hides under
    projection matmuls and the PE never idles:
      region0: A(v0,k0,q0)
      region1: A(v1,k1,q1) || B(qg0)
      region2: A(v2,k2,q2) || B(qg1)
      region3: A(v3,k3,q3) || B(qg2)   (qg2 only needs t<=2 K/V)
      tail:    B(qg3)
  - PSUM: A pool 2 banks, scores 2x[128,2,512]=4 banks, ctx [128,2,512]=2.
    bc/out-proj tiles borrow the score pool.  Total exactly 8 banks.
"""
import sys

sys.path.insert(0, "/opt/trn_rl_repo")
import numpy as np

import concourse.bacc as bacc
import concourse.tile as tile
import concourse.mybir as mybir

B, S, D = 4, 2048, 1024
H, DK = 16, 64
NCORES = 8
GCH = 512          # channels per core (8 heads)
NT = S // 128      # 16 seq chunks
F32 = mybir.dt.float32
F32R = mybir.dt.float32r
ACTF = mybir.ActivationFunctionType
ALU = mybir.AluOpType

TRACE = False          # set by test.py for profiling runs
TRACE_CORES = None
LAST_RESULT = None     # BassKernelResults stash for test.py
MM_DTYPE = "bf16"
DEBUG = False          # adds intermediate-dump DRAM outputs


def build_program(mask_mode):
    """mask_mode: 'causal' (tril), 'full' (all ones), 'general' (arbitrary)."""
    MMD = mybir.dt.bfloat16 if MM_DTYPE == "bf16" else F32R
    nc = bacc.Bacc("TRN2", target_bir_lowering=False, debug=False)

    xT = {s: nc.dram_tensor(f"x{s}T", [D, S], MMD, kind="ExternalInput")
          for s in "qkv"}
    w1T = {s: nc.dram_tensor(f"w1T_{s}", [D, GCH], MMD, kind="ExternalInput")
           for s in "qkv"}
    w2T = {s: nc.dram_tensor(f"w2T_{s}", [D, GCH], MMD, kind="ExternalInput")
           for s in "qkv"}
    bias_d = {}
    for s in "qk":
        for bn in ("b1", "b2", "b1h"):
            bias_d[f"{bn}_{s}"] = nc.dram_tensor(f"{bn}_{s}", [128, 4], F32,
                                                 kind="ExternalInput")
    b1v_d = nc.dram_tensor("b1_v", [1, GCH], MMD, kind="ExternalInput")
    b2v_d = nc.dram_tensor("b2_v", [1, GCH], MMD, kind="ExternalInput")
    woT_d = nc.dram_tensor("woT", [128, 4, D], MMD, kind="ExternalInput")
    pat_d = m01T_d = None
    if mask_mode == "causal":
        pat_d = nc.dram_tensor("pat", [128, 4, 512], MMD, kind="ExternalInput")
    elif mask_mode == "general":
        m01T_d = nc.dram_tensor("m01T", [S, S], MMD, kind="ExternalInput")
    pout_d = nc.dram_tensor("pout", [S, D], F32, kind="ExternalOutput")
    dbg = {}
    if DEBUG:
        for nm, shp in (("qt", [128, 4, S]), ("kt0", [128, 4, S]),
                        ("kt1", [128, 4, S]), ("v", [128, NT, 8, 65]),
                        ("attn", [128, 2, 512]), ("ct", [128, 4, 4, 512]),
                        ("rec", [128, 512]), ("bcw", [128, 128]),
                        ("bcsb", [128, 512])):
            dbg[nm] = nc.dram_tensor(f"dbg_{nm}", shp, MMD,
                                     kind="ExternalOutput")

    def kc_count(qg):
        return 4 * qg + 4 if mask_mode == "causal" else NT

    with tile.TileContext(nc) as tc:
        with (
            tc.tile_pool(name="persist", bufs=1) as persist,
            tc.tile_pool(name="xpool", bufs=26) as xpool,
            tc.tile_pool(name="stage", bufs=2) as stage,
            tc.tile_pool(name="apool", bufs=4) as apool,
            tc.tile_pool(name="ctpool", bufs=2) as ctpool,
            tc.tile_pool(name="cxspool", bufs=3) as cxspool,
            tc.tile_pool(name="smalls", bufs=2) as smalls,
            tc.tile_pool(name="ostage", bufs=3) as ostage,
            tc.tile_pool(name="mpool", bufs=2) as mpool,
            tc.tile_pool(name="pps", bufs=2, space="PSUM") as pps,
            tc.tile_pool(name="scps", bufs=2, space="PSUM") as scps,
            tc.tile_pool(name="cxps", bufs=1, space="PSUM") as cxps,
        ):
            # ---------------- persistent tiles + preloads ----------------
            qt_sb = persist.tile([128, 4, S], MMD, tag="qt")
            # zero-padded per-parity K: even heads live in partitions 0-63,
            # odd heads in 64-127; the other half stays zero so score
            # matmuls can contract K=128 (no PE mode switch).
            ktz = [persist.tile([128, 4, S], MMD, tag=f"ktz{p}",
                                name=f"ktz{p}")
                   for p in range(2)]
            v_sb = persist.tile([128, NT, 8, 65], MMD, tag="v")
            w1sb = {}
            w2sb = {}
            for s in "vkq":
                w1sb[s] = persist.tile([128, 8, GCH], MMD, tag=f"w1{s}",
                                       name=f"w1{s}")
                w2sb[s] = persist.tile([128, 8, GCH], MMD, tag=f"w2{s}",
                                       name=f"w2{s}")
            woT_sb = persist.tile([128, 4, D], MMD, tag="wo")

            # V biases padded to K=128 (row 0 = bias, rest zero), plus a
            # row-0-ones lhsT so the bias matmul is a full 128x128 matmul.
            b1v_sb = persist.tile([128, GCH], MMD, tag="b1v")
            b2v_sb = persist.tile([128, GCH], MMD, tag="b2v")
            onesrow = persist.tile([128, 128], MMD, tag="onesrow")
            # rec broadcast: bcW[0, 0:64] = 1, bcW[32, 64:128] = 1, rest 0.
            # bc = bcW.T @ rec_sb replicates rec rows 0/32 across partitions.
            bcW = persist.tile([128, 128], MMD, tag="bcW")
            rec_sb = persist.tile([128, 512], MMD, tag="rec")
            rec_f32 = persist.tile([33, 512], F32, tag="recf")
            den_pp = [persist.tile([33, 512], F32, tag=f"den{i}",
                                   name=f"den{i}")
                      for i in range(2)]
            nc.gpsimd.memset(den_pp[0][:], 1.0)
            nc.gpsimd.memset(den_pp[1][:], 1.0)

            nc.gpsimd.memset(ktz[0][:], 0.0)
            nc.gpsimd.memset(ktz[1][:], 0.0)
            nc.gpsimd.memset(b1v_sb[:], 0.0)
            nc.gpsimd.memset(b2v_sb[:], 0.0)
            nc.gpsimd.memset(onesrow[:], 0.0)
            nc.gpsimd.memset(onesrow[0:1, :], 1.0)
            nc.gpsimd.memset(bcW[:], 0.0)
            nc.gpsimd.memset(bcW[0:1, 0:64], 1.0)
            nc.gpsimd.memset(bcW[32:33, 64:128], 1.0)
            nc.gpsimd.memset(rec_sb[:], 0.0)
            nc.gpsimd.memset(rec_f32[:], 0.0)

            onescol = persist.tile([128, 1], F32, tag="onescol")
            nc.any.memset(onescol[:], 1.0)
            nc.vector.tensor_copy(
                v_sb[:, :, :, 64:65],
                onescol[:, None, :].to_broadcast([128, NT, 8, 1]),
            )

            # x-tile prefetch machinery (defined early so weight DMAs can
            # interleave with the first x tiles on the sync queue).
            xtiles = {}

            def prefetch_x(s, t):
                if (s, t) in xtiles:
                    return
                xs = []
                for dc in range(8):
                    xt = xpool.tile([128, 512], MMD, tag="xt")
                    nc.sync.dma_start(
                        xt[:],
                        xT[s][dc * 128:(dc + 1) * 128,
                              t * 512:(t + 1) * 512])
                    xs.append(xt)
                xtiles[(s, t)] = xs

            # weight DMAs interleaved with the x tiles each A unit needs,
            # so the first matmul starts after ~11us instead of ~50us.
            # w1_v is chunked per-dc (first matmul gates on chunk 0 only);
            # the rest go as single large rearranged DMAs (better DMA BW).
            bias_qk = {}
            for s in "vkq":
                if s == "v":
                    nc.sync.dma_start(b1v_sb[0:1, :], b1v_d[:])
                    nc.sync.dma_start(b2v_sb[0:1, :], b2v_d[:])
                prefetch_x(s, 0)
                if s == "v":
                    for dc in range(8):
                        nc.sync.dma_start(
                            w1sb[s][:, dc, :],
                            w1T[s][dc * 128:(dc + 1) * 128, :])
                else:
                    nc.sync.dma_start(
                        w1sb[s][:],
                        w1T[s][:].rearrange("(dc p) o -> p dc o", p=128))
                nc.sync.dma_start(
                    w2sb[s][:],
                    w2T[s][:].rearrange("(dc p) o -> p dc o", p=128))
                if s != "v":
                    bb = []
                    for bn in ("b1", "b2", "b1h"):
                        t_ = persist.tile([128, 4], F32, tag=f"{bn}{s}",
                                          name=f"{bn}{s}")
                        nc.sync.dma_start(t_[:], bias_d[f"{bn}_{s}"][:])
                        bb.append(t_)
                    bias_qk[s] = tuple(bb)
            pat_sb = None
            if mask_mode == "causal":
                pat_sb = persist.tile([128, 4, 512], MMD, tag="pat")
                nc.sync.dma_start(pat_sb[:], pat_d[:])
            nc.sync.dma_start(woT_sb[:], woT_d[:])

            # ---------------- phase A unit emission ----------------
            def a_subunit(s, t, jh, jj):
                """One (128 out-ch x 512 seq) SwiGLU tile: ps1 branch fully,
                stash 2*silu, then ps2 branch and write the destination."""
                j = jh * 2 + jj
                xts = xtiles[(s, t)]
                ps1 = pps.tile([128, 512], F32, tag="pp", name="ps1")
                for dc in range(8):
                    if s == "v":
                        nc.tensor.matmul(
                            ps1[:], xts[dc][:, j * 128:(j + 1) * 128],
                            w1sb[s][:, dc, :],
                            start=(dc == 0), stop=False)
                    else:
                        nc.tensor.matmul(
                            ps1[:], w1sb[s][:, dc, j * 128:(j + 1) * 128],
                            xts[dc][:],
                            start=(dc == 0), stop=(dc == 7))
                if s == "v":
                    nc.tensor.matmul(ps1[:], onesrow[:], b1v_sb[:],
                                     start=False, stop=True)
                act = stage.tile([128, 512], F32, tag="act")
                silu = stage.tile([128, 512], F32, tag="silu")
                if s == "v":
                    # act = tanh(ps1/2); silu2 = ps1*(1+act) = 2*silu(ps1)
                    nc.scalar.activation(act[:], ps1[:], ACTF.Tanh, scale=0.5)
                    nc.vector.scalar_tensor_tensor(
                        silu[:], act[:], 1.0, ps1[:],
                        op0=ALU.add, op1=ALU.mult)
                else:
                    b1_, b2_, b1h_ = bias_qk[s]
                    nc.scalar.activation(
                        act[:], ps1[:], ACTF.Tanh,
                        scale=0.5, bias=b1h_[:, j:j + 1])
                    a_sb = stage.tile([128, 512], F32, tag="asb")
                    nc.vector.tensor_scalar_add(a_sb[:], ps1[:],
                                                b1_[:, j:j + 1])
                    nc.vector.scalar_tensor_tensor(
                        silu[:], act[:], 1.0, a_sb[:],
                        op0=ALU.add, op1=ALU.mult)
                ps2 = pps.tile([128, 512], F32, tag="pp", name="ps2")
                for dc in range(8):
                    if s == "v":
                        nc.tensor.matmul(
                            ps2[:], xts[dc][:, j * 128:(j + 1) * 128],
                            w2sb[s][:, dc, :],
                            start=(dc == 0), stop=False)
                    else:
                        nc.tensor.matmul(
                            ps2[:], w2sb[s][:, dc, j * 128:(j + 1) * 128],
                            xts[dc][:],
                            start=(dc == 0), stop=(dc == 7))
                if s == "v":
                    nc.tensor.matmul(ps2[:], onesrow[:], b2v_sb[:],
                                     start=False, stop=True)
                    nt_i = t * 4 + j
                    nc.vector.tensor_tensor(
                        v_sb[:, nt_i, :, 0:64],
                        ps2[:].rearrange("p (h d) -> p h d", h=8),
                        silu[:].rearrange("p (h d) -> p h d", h=8),
                        ALU.mult)
                elif s == "q":
                    nc.vector.scalar_tensor_tensor(
                        qt_sb[:, j, t * 512:(t + 1) * 512],
                        ps2[:], bias_qk["q"][1][:, j:j + 1], silu[:],
                        op0=ALU.add, op1=ALU.mult)
                else:
                    # K: split into per-parity zero-padded tensors
                    for par in range(2):
                        bp = par * 64
                        nc.vector.scalar_tensor_tensor(
                            ktz[par][bp:bp + 64, j, t * 512:(t + 1) * 512],
                            ps2[bp:bp + 64, :],
                            bias_qk["k"][1][bp:bp + 64, j:j + 1],
                            silu[bp:bp + 64, :],
                            op0=ALU.add, op1=ALU.mult)

            A_SUB_COST = {"v": 3900, "k": 3700, "q": 3600}

            def a_unit_items(s, t):
                items = []
                for jh in range(2):
                    for jj in range(2):
                        items.append((A_SUB_COST[s],
                                      lambda s=s, t=t, jh=jh, jj=jj:
                                      a_subunit(s, t, jh, jj)))
                return items

            # ---------------- phase B unit emission ----------------
            mtiles = {}

            def prefetch_mask(qg):
                if mask_mode != "general" or qg in mtiles:
                    return
                mt_sb = mpool.tile([128, NT, 512], MMD, tag="mt")
                qsl = slice(qg * 512, (qg + 1) * 512)
                for kc in range(kc_count(qg)):
                    nc.sync.dma_start(
                        mt_sb[:, kc, :],
                        m01T_d[kc * 128:(kc + 1) * 128, qsl])
                mtiles[qg] = mt_sb

            ct_tiles = {}

            def ctx_ap(ctx, rows, par):
                if isinstance(ctx, tuple):
                    return ctx[par][rows, :]
                return ctx[rows, par, :]

            def b_kc_unit(qg, pj, kc, ctx, kcmax):
                # causal diagonal blocks: q columns below the block's first
                # unmasked row are entirely masked -> restrict the score
                # matmul / exp / mask / AV to the live q range.  The dead
                # columns are never touched (AV skips them too), so junk in
                # the attn tile there is never consumed.
                lo = 0
                if mask_mode == "causal" and kc > 4 * qg:
                    lo = (kc - 4 * qg) * 128
                nq = 512 - lo
                qsl = slice(qg * 512 + lo, (qg + 1) * 512)
                ksl = slice(kc * 128, (kc + 1) * 128)
                sc = scps.tile([128, 2, 512], F32, tag="sc", name="sc")
                for par in range(2):
                    nc.tensor.matmul(
                        sc[:, par, lo:512],
                        ktz[par][:, pj, ksl],
                        qt_sb[:, pj, qsl])
                attn = apool.tile([128, 2, 512], MMD, tag="at")
                if lo:
                    nc.scalar.activation(attn[:, :, lo:512],
                                         sc[:, :, lo:512], ACTF.Exp)
                else:
                    nc.scalar.activation(attn[:], sc[:], ACTF.Exp)
                if DEBUG and (qg, pj, kc) == (0, 0, 0):
                    nc.sync.dma_start(dbg["attn"][:], attn[:])
                if mask_mode == "causal" and kc >= 4 * qg:
                    nc.vector.tensor_tensor(
                        attn[:, :, lo:512], attn[:, :, lo:512],
                        pat_sb[:, kc - 4 * qg, None, lo:512].to_broadcast(
                            [128, 2, nq]),
                        ALU.mult)
                elif mask_mode == "general":
                    nc.vector.tensor_tensor(
                        attn[:], attn[:],
                        mtiles[qg][:, kc, None, :].to_broadcast(
                            [128, 2, 512]),
                        ALU.mult)
                for par in range(2):
                    hl = 2 * pj + par
                    nc.tensor.matmul(
                        ctx_ap(ctx, slice(0, 65), par)[:, lo:512],
                        v_sb[:, kc, hl, :],
                        attn[:, par, lo:512],
                        start=(kc == 0),
                        stop=(kc == kcmax - 1))

            def b_evict_unit(qg, pj, ctx, cxs_box):
                # short critical chain: pull denominators + context out of
                # PSUM so the single ctx buffer frees for the next pj.
                den = den_pp[pj % 2]
                for par in range(2):
                    nc.vector.tensor_copy(den[32 * par:32 * par + 1, :],
                                          ctx_ap(ctx, slice(64, 65), par))
                cxs = cxspool.tile([128, 512], MMD, tag="cxs",
                                   name=f"cxs{qg}_{pj}")
                for par in range(2):
                    nc.vector.tensor_copy(cxs[64 * par:64 * par + 64, :],
                                          ctx_ap(ctx, slice(0, 64), par))
                cxs_box[0] = cxs

            def b_norm_unit(qg, pj, cxs_box):
                # off the critical path: reciprocal, partition-broadcast via
                # bcW matmul, and the normalizing multiply from SBUF staging.
                den = den_pp[pj % 2]
                if mask_mode == "general":
                    nc.vector.reciprocal(rec_f32[:], den[:])
                else:
                    nc.vector.reciprocal_approx_fast(rec_f32[:], den[:])
                nc.vector.tensor_copy(rec_sb[0:33, :], rec_f32[:])
                if DEBUG and (qg, pj) == (0, 0):
                    nc.sync.dma_start(dbg["rec"][:], rec_sb[:])
                bc = scps.tile([128, 2, 512], F32, tag="sc", name="bc")
                nc.tensor.matmul(bc[:, 0, :], bcW[:], rec_sb[:])
                bc_sb = smalls.tile([128, 512], F32, tag="bcs")
                nc.vector.tensor_copy(bc_sb[:], bc[:, 0, :])
                if DEBUG and (qg, pj) == (0, 0):
                    nc.sync.dma_start(dbg["bcw"][:], bcW[:])
                    bcs16 = smalls.tile([128, 512], MMD, tag="bcs16")
                    nc.vector.tensor_copy(bcs16[:], bc_sb[:])
                    nc.sync.dma_start(dbg["bcsb"][:], bcs16[:])
                ct_qg = ct_tiles[qg]
                nc.vector.tensor_tensor(
                    ct_qg[:, pj, :], cxs_box[0][:], bc_sb[:], ALU.mult)

            def b_out_unit(qg, ns, oh):
                nt_i = qg * 4 + ns
                nsl = slice(ns * 128, (ns + 1) * 128)
                ct_qg = ct_tiles[qg]
                po = scps.tile([128, 2, 512], F32, tag="sc", name="po")
                for j in range(4):
                    nc.tensor.matmul(
                        po[:, 0, :],
                        ct_qg[:, j, nsl],
                        woT_sb[:, j, oh * 512:(oh + 1) * 512],
                        start=(j == 0), stop=(j == 3))
                ot = ostage.tile([128, 512], F32, tag="ot")
                nc.vector.tensor_copy(ot[:], po[:, 0, :])
                nc.sync.dma_start(
                    pout_d[nt_i * 128:(nt_i + 1) * 128,
                           oh * 512:(oh + 1) * 512],
                    ot[:])

            def b_qg_items(qg):
                prefetch_mask(qg)
                kcmax = kc_count(qg)
                items = []

                def start_qg(qg=qg):
                    ct_tiles[qg] = ctpool.tile([128, 4, 512], MMD, tag="ct",
                                               name=f"ct{qg}")
                items.append((0, start_qg))
                pending_norm = None
                for pj in range(4):
                    ctx_box = {}

                    def start_pj(ctx_box=ctx_box, qg=qg, pj=pj):
                        if qg == 3 and pj % 2 == 1:
                            # A-phase PSUM banks are free by now; use them
                            # as a second ctx buffer to pipeline pj's.
                            ca = pps.tile([128, 512], F32, tag="pp",
                                          name=f"cxa{qg}_{pj}")
                            cb = pps.tile([128, 512], F32, tag="pp",
                                          name=f"cxb{qg}_{pj}")
                            ctx_box[0] = (ca, cb)
                        else:
                            ctx_box[0] = cxps.tile([128, 2, 512], F32,
                                                   tag="cx",
                                                   name=f"cx{qg}_{pj}")
                    items.append((0, start_pj))
                    for kc in range(kcmax):
                        items.append((900,
                                      lambda qg=qg, pj=pj, kc=kc,
                                      ctx_box=ctx_box, kcmax=kcmax:
                                      b_kc_unit(qg, pj, kc, ctx_box[0],
                                                kcmax)))
                        # the deferred norm of the previous pj goes a few kc
                        # units in, so its bc matmul never heads the PE FIFO
                        # while its DVE reciprocal chain is still running.
                        if kc == 2 and pending_norm is not None:
                            items.append(pending_norm)
                            pending_norm = None
                    cxs_box = {}
                    items.append((100,
                                  lambda qg=qg, pj=pj, ctx_box=ctx_box,
                                  cxs_box=cxs_box:
                                  b_evict_unit(qg, pj, ctx_box[0], cxs_box)))
                    pending_norm = (300,
                                    lambda qg=qg, pj=pj, cxs_box=cxs_box:
                                    b_norm_unit(qg, pj, cxs_box))
                if pending_norm is not None:
                    items.append(pending_norm)
                    pending_norm = None
                if DEBUG:
                    def dump_ct(qg=qg):
                        nc.sync.dma_start(dbg["ct"][:, qg, :, :],
                                          ct_tiles[qg][:])
                    items.append((0, dump_ct))
                return items

            def b_out_items(qg):
                items = []
                for ns in range(4):
                    for oh in range(2):
                        items.append((900,
                                      lambda qg=qg, ns=ns, oh=oh:
                                      b_out_unit(qg, ns, oh)))
                return items

            # ---------------- interleaved schedule ----------------
            def interleave(a_items, b_items):
                ta = sum(c for c, _ in a_items) or 1
                tb = sum(c for c, _ in b_items) or 1
                ca = cb = 0
                ia = ib = 0
                while ia < len(a_items) or ib < len(b_items):
                    if ib >= len(b_items) or (
                            ia < len(a_items) and ca * tb <= cb * ta):
                        c, f = a_items[ia]
                        ia += 1
                        ca += c
                    else:
                        c, f = b_items[ib]
                        ib += 1
                        cb += c
                    f()

            regions = [
                ([("v", 0), ("k", 0), ("q", 0)], []),
                ([("v", 1), ("k", 1), ("q", 1)], [0]),
                ([("v", 2), ("k", 2), ("q", 2)], [1]),
                ([("v", 3), ("k", 3), ("q", 3)], [2]),
                ([], [3]),
            ]
            # prefetch x for the first region up front
            for s, t in regions[0][0]:
                prefetch_x(s, t)
            for ri, (aunits, bqgs) in enumerate(regions):
                # prefetch next region's x tiles (overlaps this region)
                if ri + 1 < len(regions):
                    for s, t in regions[ri + 1][0]:
                        prefetch_x(s, t)
                a_items = [it for (s, t) in aunits for it in a_unit_items(s, t)]
                b_items = []
                for qg in bqgs:
                    b_items += b_qg_items(qg) + b_out_items(qg)
                interleave(a_items, b_items)
            if DEBUG:
                nc.sync.dma_start(dbg["qt"][:], qt_sb[:])
                nc.sync.dma_start(dbg["kt0"][:], ktz[0][:])
                nc.sync.dma_start(dbg["kt1"][:], ktz[1][:])
                nc.sync.dma_start(dbg["v"][:], v_sb[:])
    nc.compile()
    return nc


def _host_prepare(inputs):
    """Split the full problem into 8 per-core input maps + host-side info."""
    q = np.asarray(inputs["query"], dtype=np.float32)
    k = np.asarray(inputs["key"], dtype=np.float32)
    v = np.asarray(inputs["value"], dtype=np.float32)
    mask = np.asarray(inputs["mask"])
    w = {n: np.asarray(inputs[n], dtype=np.float32)
         for n in ("wq1", "wq2", "wk1", "wk2", "wv1", "wv2", "wo")}
    bias = {n: np.asarray(inputs[n], dtype=np.float32)
            for n in ("bq1", "bq2", "bk1", "bk2", "bv1", "bv2", "bo")}

    m = mask.reshape(S, S)
    if np.array_equal(m != 0, np.tril(np.ones((S, S), bool))):
        mask_mode = "causal"
    elif np.all(m != 0):
        mask_mode = "full"
    else:
        mask_mode = "general"

    pat = None
    m01T = None
    if mask_mode == "causal":
        kk = np.arange(128)[:, None]
        qq = np.arange(512)[None, :]
        pat = np.stack(
            [(kk + 128 * i <= qq).astype(np.float32) for i in range(4)], axis=1
        )  # [128, 4, 512]
        pat = np.ascontiguousarray(pat)
    elif mask_mode == "general":
        m01T = np.ascontiguousarray((m != 0).T.astype(np.float32))

    scale = 1.0 / np.sqrt(DK).astype(np.float32)

    if MM_DTYPE == "bf16":
        import ml_dtypes

        mmd_np = ml_dtypes.bfloat16
    else:
        mmd_np = np.float32

    def cvt(a):
        return np.ascontiguousarray(a).astype(mmd_np)

    in_maps = []
    for c in range(NCORES):
        b, g = divmod(c, 2)
        sl = slice(g * GCH, (g + 1) * GCH)
        im = {
            "xqT": cvt(q[b].T),
            "xkT": cvt(k[b].T),
            "xvT": cvt(v[b].T),
            "w1T_q": cvt(w["wq1"][sl].T),
            # fold the 1/sqrt(dk) score scale into the non-silu Q branch,
            # and 0.5 everywhere (silu computed as A*(1+tanh(A/2)) = 2*silu)
            "w2T_q": cvt(w["wq2"][sl].T * (scale * 0.5)),
            "w2T_k": cvt(w["wk2"][sl].T * 0.5),
            "w2T_v": cvt(w["wv2"][sl].T * 0.5),
            "w1T_k": cvt(w["wk1"][sl].T),
            "w1T_v": cvt(w["wv1"][sl].T),
            "b1_q": np.ascontiguousarray(bias["bq1"][sl].reshape(4, 128).T),
            "b1h_q": np.ascontiguousarray(
                (bias["bq1"][sl] * 0.5).reshape(4, 128).T),
            "b2_q": np.ascontiguousarray(
                (bias["bq2"][sl] * (scale * 0.5)).reshape(4, 128).T),
            "b1_k": np.ascontiguousarray(bias["bk1"][sl].reshape(4, 128).T),
            "b1h_k": np.ascontiguousarray(
                (bias["bk1"][sl] * 0.5).reshape(4, 128).T),
            "b2_k": np.ascontiguousarray(
                (bias["bk2"][sl] * 0.5).reshape(4, 128).T),
            "b1_v": cvt(bias["bv1"][sl].reshape(1, GCH)),
            "b2_v": cvt((bias["bv2"][sl] * 0.5).reshape(1, GCH)),
            "woT": cvt(
                w["wo"][:, sl].T.reshape(4, 128, D).transpose(1, 0, 2)),
        }
        if mask_mode == "causal":
            im["pat"] = cvt(pat)
        elif mask_mode == "general":
            im["m01T"] = cvt(m01T)
        in_maps.append(im)
    return mask_mode, in_maps, bias["bo"]


def kernel(**inputs):
    global LAST_RESULT
    mask_mode, in_maps, bo = _host_prepare(inputs)
    nc = build_program(mask_mode)

    import concourse.bass_utils as bu

    if TRACE:
        import types

        try:
            from trn_agent_boot.trn_boot import _ntff_profile_via_ctypes

            hook = _ntff_profile_via_ctypes("/opt/axon/libaxon_pjrt.so")
            m = types.ModuleType("antenv.axon_hooks")
            m.get_axon_ntff_profile_hook = lambda: hook
            import antenv  # noqa: F401

            sys.modules["antenv.axon_hooks"] = m
            bu.upload_artifacts = lambda d: "local://skipped"
        except Exception as e:
            print("profiling hook install failed:", e)

    res = bu.run_bass_kernel_spmd(
        nc, in_maps, core_ids=list(range(NCORES)),
        trace=TRACE, trace_cores=TRACE_CORES,
    )
    LAST_RESULT = res

    out = np.empty((B, S, D), dtype=np.float32)
    for b in range(B):
        out[b] = (res.results[2 * b]["pout"] + res.results[2 * b + 1]["pout"]
                  + bo[None, :])
    return out


# revision 49
# speedup vs baseline: 1.1875x; 1.0129x over previous
"""SwiGLU-projected causal MHA (B=4, S=2048, D=1024, H=16) on 8 TRN2 NeuronCores.

Sharding: core c -> (batch b = c//2, head-group g = c%2).  Each core computes
the SwiGLU Q/K/V projections for its 512 output channels (= 8 heads) of its
batch, runs causal attention for those heads, and produces a partial output
projection (contraction over its 512 channels).  The host sums the two
partials per batch and adds the output bias.

Key structure (v2):
  - Every matmul is 128x128 mode (K padded to 128 via zeroed weight halves,
    bias matmuls padded with zero rows) -> zero PE mode-switch drains.
  - K tensor stored per-head-parity zero-padded (ktz_e/ktz_o) so score
    matmuls contract K=128 with the other head's rows zeroed.
  - exp batched over [128, 2, 512] PSUM pairs (both heads of a pair per kc).
  - softmax denominator comes out of the AV matmul (ones column in V);
    reciprocal via DVE reciprocal_approx_fast; broadcast across partitions
    via one K=128 matmul with a constant indicator matrix.
  - Phase A (projections) and phase B (attention) are interleaved in
    emission order, paced by estimated PE cost, so exp/DVE work hides under
    projection matmuls and the PE never idles:
      region0: A(v0,k0,q0)
      region1: A(v1,k1,q1) || B(qg0)
      region2: A(v2,k2,q2) || B(qg1)
      region3: A(v3,k3,q3) || B(qg2)   (qg2 only needs t<=2 K/V)
      tail:    B(qg3)
  - PSUM: A pool 2 banks, scores 2x[128,2,512]=4 banks, ctx [128,2,512]=2.
    bc/out-proj tiles borrow the score pool.  Total exactly 8 banks.
"""
import sys

sys.path.insert(0, "/opt/trn_rl_repo")
import numpy as np

import concourse.bacc as bacc
import concourse.tile as tile
import concourse.mybir as mybir

B, S, D = 4, 2048, 1024
H, DK = 16, 64
NCORES = 8
GCH = 512          # channels per core (8 heads)
NT = S // 128      # 16 seq chunks
F32 = mybir.dt.float32
F32R = mybir.dt.float32r
ACTF = mybir.ActivationFunctionType
ALU = mybir.AluOpType

TRACE = False          # set by test.py for profiling runs
TRACE_CORES = None
LAST_RESULT = None     # BassKernelResults stash for test.py
MM_DTYPE = "bf16"
DEBUG = False          # adds intermediate-dump DRAM outputs


def build_program(mask_mode):
    """mask_mode: 'causal' (tril), 'full' (all ones), 'general' (arbitrary)."""
    MMD = mybir.dt.bfloat16 if MM_DTYPE == "bf16" else F32R
    nc = bacc.Bacc("TRN2", target_bir_lowering=False, debug=False)

    xT = {s: nc.dram_tensor(f"x{s}T", [D, S], MMD, kind="ExternalInput")
          for s in "qkv"}
    w1T = {s: nc.dram_tensor(f"w1T_{s}", [D, GCH], MMD, kind="ExternalInput")
           for s in "qkv"}
    w2T = {s: nc.dram_tensor(f"w2T_{s}", [D, GCH], MMD, kind="ExternalInput")
           for s in "qkv"}
    bias_d = {}
    for s in "qk":
        for bn in ("b1", "b2", "b1h"):
            bias_d[f"{bn}_{s}"] = nc.dram_tensor(f"{bn}_{s}", [128, 4], F32,
                                                 kind="ExternalInput")
    b1v_d = nc.dram_tensor("b1_v", [1, GCH], MMD, kind="ExternalInput")
    b2v_d = nc.dram_tensor("b2_v", [1, GCH], MMD, kind="ExternalInput")
    woT_d = nc.dram_tensor("woT", [128, 4, D], MMD, kind="ExternalInput")
    pat_d = m01T_d = None
    if mask_mode == "causal":
        pat_d = nc.dram_tensor("pat", [128, 4, 512], MMD, kind="ExternalInput")
    elif mask_mode == "general":
        m01T_d = nc.dram_tensor("m01T", [S, S], MMD, kind="ExternalInput")
    pout_d = nc.dram_tensor("pout", [S, D], F32, kind="ExternalOutput")
    dbg = {}
    if DEBUG:
        for nm, shp in (("qt", [128, 4, S]), ("kt0", [128, 4, S]),
                        ("kt1", [128, 4, S]), ("v", [128, NT, 8, 65]),
                        ("attn", [128, 2, 512]), ("ct", [128, 4, 4, 512]),
                        ("rec", [128, 512]), ("bcw", [128, 128]),
                        ("bcsb", [128, 512])):
            dbg[nm] = nc.dram_tensor(f"dbg_{nm}", shp, MMD,
                                     kind="ExternalOutput")

    def kc_count(qg):
        return 4 * qg + 4 if mask_mode == "causal" else NT

    with tile.TileContext(nc) as tc:
        with (
            tc.tile_pool(name="persist", bufs=1) as persist,
            tc.tile_pool(name="xpool", bufs=32) as xpool,
            tc.tile_pool(name="stage", bufs=2) as stage,
            tc.tile_pool(name="apool", bufs=4) as apool,
            tc.tile_pool(name="ctpool", bufs=2) as ctpool,
            tc.tile_pool(name="cxspool", bufs=3) as cxspool,
            tc.tile_pool(name="smalls", bufs=2) as smalls,
            tc.tile_pool(name="ostage", bufs=3) as ostage,
            tc.tile_pool(name="mpool", bufs=2) as mpool,
            tc.tile_pool(name="pps", bufs=2, space="PSUM") as pps,
            tc.tile_pool(name="scps", bufs=2, space="PSUM") as scps,
            tc.tile_pool(name="cxps", bufs=1, space="PSUM") as cxps,
        ):
            # ---------------- persistent tiles + preloads ----------------
            qt_sb = persist.tile([128, 4, S], MMD, tag="qt")
            # zero-padded per-parity K: even heads live in partitions 0-63,
            # odd heads in 64-127; the other half stays zero so score
            # matmuls can contract K=128 (no PE mode switch).
            ktz = [persist.tile([128, 4, S], MMD, tag=f"ktz{p}",
                                name=f"ktz{p}")
                   for p in range(2)]
            v_sb = persist.tile([128, NT, 8, 65], MMD, tag="v")
            w1sb = {}
            w2sb = {}
            for s in "vkq":
                w1sb[s] = persist.tile([128, 8, GCH], MMD, tag=f"w1{s}",
                                       name=f"w1{s}")
                w2sb[s] = persist.tile([128, 8, GCH], MMD, tag=f"w2{s}",
                                       name=f"w2{s}")
            woT_sb = persist.tile([128, 4, D], MMD, tag="wo")

            # V biases padded to K=128 (row 0 = bias, rest zero), plus a
            # row-0-ones lhsT so the bias matmul is a full 128x128 matmul.
            b1v_sb = persist.tile([128, GCH], MMD, tag="b1v")
            b2v_sb = persist.tile([128, GCH], MMD, tag="b2v")
            onesrow = persist.tile([128, 128], MMD, tag="onesrow")
            # rec broadcast: bcW[0, 0:64] = 1, bcW[32, 64:128] = 1, rest 0.
            # bc = bcW.T @ rec_sb replicates rec rows 0/32 across partitions.
            bcW = persist.tile([128, 128], MMD, tag="bcW")
            rec_sb = persist.tile([128, 512], MMD, tag="rec")
            rec_f32 = persist.tile([33, 512], F32, tag="recf")
            den_pp = [persist.tile([33, 512], F32, tag=f"den{i}",
                                   name=f"den{i}")
                      for i in range(2)]
            nc.gpsimd.memset(den_pp[0][:], 1.0)
            nc.gpsimd.memset(den_pp[1][:], 1.0)

            nc.gpsimd.memset(ktz[0][:], 0.0)
            nc.gpsimd.memset(ktz[1][:], 0.0)
            nc.gpsimd.memset(b1v_sb[:], 0.0)
            nc.gpsimd.memset(b2v_sb[:], 0.0)
            nc.gpsimd.memset(onesrow[:], 0.0)
            nc.gpsimd.memset(onesrow[0:1, :], 1.0)
            nc.gpsimd.memset(bcW[:], 0.0)
            nc.gpsimd.memset(bcW[0:1, 0:64], 1.0)
            nc.gpsimd.memset(bcW[32:33, 64:128], 1.0)
            nc.gpsimd.memset(rec_sb[:], 0.0)
            nc.gpsimd.memset(rec_f32[:], 0.0)

            onescol = persist.tile([128, 1], F32, tag="onescol")
            nc.any.memset(onescol[:], 1.0)
            nc.vector.tensor_copy(
                v_sb[:, :, :, 64:65],
                onescol[:, None, :].to_broadcast([128, NT, 8, 1]),
            )

            # x-tile prefetch machinery (defined early so weight DMAs can
            # interleave with the first x tiles on the sync queue).
            xtiles = {}

            def prefetch_x(s, t):
                if (s, t) in xtiles:
                    return
                xs = []
                for dc in range(8):
                    xt = xpool.tile([128, 512], MMD, tag="xt")
                    nc.sync.dma_start(
                        xt[:],
                        xT[s][dc * 128:(dc + 1) * 128,
                              t * 512:(t + 1) * 512])
                    xs.append(xt)
                xtiles[(s, t)] = xs

            # weight DMAs interleaved with the x tiles each A unit needs,
            # so the first matmul starts after ~11us instead of ~50us.
            # w1_v is chunked per-dc (first matmul gates on chunk 0 only);
            # the rest go as single large rearranged DMAs (better DMA BW).
            bias_qk = {}
            for s in "vkq":
                if s == "v":
                    nc.sync.dma_start(b1v_sb[0:1, :], b1v_d[:])
                    nc.sync.dma_start(b2v_sb[0:1, :], b2v_d[:])
                prefetch_x(s, 0)
                if s == "v":
                    for dc in range(8):
                        nc.sync.dma_start(
                            w1sb[s][:, dc, :],
                            w1T[s][dc * 128:(dc + 1) * 128, :])
                else:
                    nc.sync.dma_start(
                        w1sb[s][:],
                        w1T[s][:].rearrange("(dc p) o -> p dc o", p=128))
                nc.sync.dma_start(
                    w2sb[s][:],
                    w2T[s][:].rearrange("(dc p) o -> p dc o", p=128))
                if s != "v":
                    bb = []
                    for bn in ("b1", "b2", "b1h"):
                        t_ = persist.tile([128, 4], F32, tag=f"{bn}{s}",
                                          name=f"{bn}{s}")
                        nc.sync.dma_start(t_[:], bias_d[f"{bn}_{s}"][:])
                        bb.append(t_)
                    bias_qk[s] = tuple(bb)
            pat_sb = None
            if mask_mode == "causal":
                pat_sb = persist.tile([128, 4, 512], MMD, tag="pat")
                nc.sync.dma_start(pat_sb[:], pat_d[:])
            nc.sync.dma_start(woT_sb[:], woT_d[:])

            # ---------------- phase A unit emission ----------------
            def a_subunit(s, t, jh, jj):
                """One (128 out-ch x 512 seq) SwiGLU tile: ps1 branch fully,
                stash 2*silu, then ps2 branch and write the destination."""
                j = jh * 2 + jj
                xts = xtiles[(s, t)]
                ps1 = pps.tile([128, 512], F32, tag="pp", name="ps1")
                for dc in range(8):
                    if s == "v":
                        nc.tensor.matmul(
                            ps1[:], xts[dc][:, j * 128:(j + 1) * 128],
                            w1sb[s][:, dc, :],
                            start=(dc == 0), stop=False)
                    else:
                        nc.tensor.matmul(
                            ps1[:], w1sb[s][:, dc, j * 128:(j + 1) * 128],
                            xts[dc][:],
                            start=(dc == 0), stop=(dc == 7))
                if s == "v":
                    nc.tensor.matmul(ps1[:], onesrow[:], b1v_sb[:],
                                     start=False, stop=True)
                act = stage.tile([128, 512], F32, tag="act")
                silu = stage.tile([128, 512], F32, tag="silu")
                if s == "v":
                    # act = tanh(ps1/2); silu2 = ps1*(1+act) = 2*silu(ps1)
                    nc.scalar.activation(act[:], ps1[:], ACTF.Tanh, scale=0.5)
                    nc.vector.scalar_tensor_tensor(
                        silu[:], act[:], 1.0, ps1[:],
                        op0=ALU.add, op1=ALU.mult)
                else:
                    b1_, b2_, b1h_ = bias_qk[s]
                    nc.scalar.activation(
                        act[:], ps1[:], ACTF.Tanh,
                        scale=0.5, bias=b1h_[:, j:j + 1])
                    a_sb = stage.tile([128, 512], F32, tag="asb")
                    nc.vector.tensor_scalar_add(a_sb[:], ps1[:],
                                                b1_[:, j:j + 1])
                    nc.vector.scalar_tensor_tensor(
                        silu[:], act[:], 1.0, a_sb[:],
                        op0=ALU.add, op1=ALU.mult)
                ps2 = pps.tile([128, 512], F32, tag="pp", name="ps2")
                for dc in range(8):
                    if s == "v":
                        nc.tensor.matmul(
                            ps2[:], xts[dc][:, j * 128:(j + 1) * 128],
                            w2sb[s][:, dc, :],
                            start=(dc == 0), stop=False)
                    else:
                        nc.tensor.matmul(
                            ps2[:], w2sb[s][:, dc, j * 128:(j + 1) * 128],
                            xts[dc][:],
                            start=(dc == 0), stop=(dc == 7))
                if s == "v":
                    nc.tensor.matmul(ps2[:], onesrow[:], b2v_sb[:],
                                     start=False, stop=True)
                    nt_i = t * 4 + j
                    nc.vector.tensor_tensor(
                        v_sb[:, nt_i, :, 0:64],
                        ps2[:].rearrange("p (h d) -> p h d", h=8),
                        silu[:].rearrange("p (h d) -> p h d", h=8),
                        ALU.mult)
                elif s == "q":
                    nc.vector.scalar_tensor_tensor(
                        qt_sb[:, j, t * 512:(t + 1) * 512],
                        ps2[:], bias_qk["q"][1][:, j:j + 1], silu[:],
                        op0=ALU.add, op1=ALU.mult)
                else:
                    # K: split into per-parity zero-padded tensors
                    for par in range(2):
                        bp = par * 64
                        nc.vector.scalar_tensor_tensor(
                            ktz[par][bp:bp + 64, j, t * 512:(t + 1) * 512],
                            ps2[bp:bp + 64, :],
                            bias_qk["k"][1][bp:bp + 64, j:j + 1],
                            silu[bp:bp + 64, :],
                            op0=ALU.add, op1=ALU.mult)

            A_SUB_COST = {"v": 3900, "k": 3700, "q": 3600}

            def a_unit_items(s, t):
                items = []
                for jh in range(2):
                    for jj in range(2):
                        items.append((A_SUB_COST[s],
                                      lambda s=s, t=t, jh=jh, jj=jj:
                                      a_subunit(s, t, jh, jj)))
                return items

            # ---------------- phase B unit emission ----------------
            mtiles = {}

            def prefetch_mask(qg):
                if mask_mode != "general" or qg in mtiles:
                    return
                mt_sb = mpool.tile([128, NT, 512], MMD, tag="mt")
                qsl = slice(qg * 512, (qg + 1) * 512)
                for kc in range(kc_count(qg)):
                    nc.sync.dma_start(
                        mt_sb[:, kc, :],
                        m01T_d[kc * 128:(kc + 1) * 128, qsl])
                mtiles[qg] = mt_sb

            ct_tiles = {}

            def ctx_ap(ctx, rows, par):
                if isinstance(ctx, tuple):
                    return ctx[par][rows, :]
                return ctx[rows, par, :]

            def b_kc_unit(qg, pj, kc, ctx, kcmax):
                # causal diagonal blocks: q columns below the block's first
                # unmasked row are entirely masked -> restrict the score
                # matmul / exp / mask / AV to the live q range.  The dead
                # columns are never touched (AV skips them too), so junk in
                # the attn tile there is never consumed.
                lo = 0
                if mask_mode == "causal" and kc > 4 * qg:
                    lo = (kc - 4 * qg) * 128
                nq = 512 - lo
                qsl = slice(qg * 512 + lo, (qg + 1) * 512)
                ksl = slice(kc * 128, (kc + 1) * 128)
                sc = scps.tile([128, 2, 512], F32, tag="sc", name="sc")
                for par in range(2):
                    nc.tensor.matmul(
                        sc[:, par, lo:512],
                        ktz[par][:, pj, ksl],
                        qt_sb[:, pj, qsl])
                attn = apool.tile([128, 2, 512], MMD, tag="at")
                if lo:
                    nc.scalar.activation(attn[:, :, lo:512],
                                         sc[:, :, lo:512], ACTF.Exp)
                else:
                    nc.scalar.activation(attn[:], sc[:], ACTF.Exp)
                if DEBUG and (qg, pj, kc) == (0, 0, 0):
                    nc.sync.dma_start(dbg["attn"][:], attn[:])
                if mask_mode == "causal" and kc >= 4 * qg:
                    nc.vector.tensor_tensor(
                        attn[:, :, lo:512], attn[:, :, lo:512],
                        pat_sb[:, kc - 4 * qg, None, lo:512].to_broadcast(
                            [128, 2, nq]),
                        ALU.mult)
                elif mask_mode == "general":
                    nc.vector.tensor_tensor(
                        attn[:], attn[:],
                        mtiles[qg][:, kc, None, :].to_broadcast(
                            [128, 2, 512]),
                        ALU.mult)
                for par in range(2):
                    hl = 2 * pj + par
                    nc.tensor.matmul(
                        ctx_ap(ctx, slice(0, 65), par)[:, lo:512],
                        v_sb[:, kc, hl, :],
                        attn[:, par, lo:512],
                        start=(kc == 0),
                        stop=(kc == kcmax - 1))

            def b_evict_unit(qg, pj, ctx, cxs_box):
                # short critical chain: pull denominators + context out of
                # PSUM so the single ctx buffer frees for the next pj.
                den = den_pp[pj % 2]
                for par in range(2):
                    nc.vector.tensor_copy(den[32 * par:32 * par + 1, :],
                                          ctx_ap(ctx, slice(64, 65), par))
                cxs = cxspool.tile([128, 512], MMD, tag="cxs",
                                   name=f"cxs{qg}_{pj}")
                for par in range(2):
                    nc.vector.tensor_copy(cxs[64 * par:64 * par + 64, :],
                                          ctx_ap(ctx, slice(0, 64), par))
                cxs_box[0] = cxs

            def b_norm_unit(qg, pj, cxs_box):
                # off the critical path: reciprocal, partition-broadcast via
                # bcW matmul, and the normalizing multiply from SBUF staging.
                den = den_pp[pj % 2]
                if mask_mode == "general":
                    nc.vector.reciprocal(rec_f32[:], den[:])
                else:
                    nc.vector.reciprocal_approx_fast(rec_f32[:], den[:])
                nc.vector.tensor_copy(rec_sb[0:33, :], rec_f32[:])
                if DEBUG and (qg, pj) == (0, 0):
                    nc.sync.dma_start(dbg["rec"][:], rec_sb[:])
                bc = scps.tile([128, 2, 512], F32, tag="sc", name="bc")
                nc.tensor.matmul(bc[:, 0, :], bcW[:], rec_sb[:])
                bc_sb = smalls.tile([128, 512], F32, tag="bcs")
                nc.vector.tensor_copy(bc_sb[:], bc[:, 0, :])
                if DEBUG and (qg, pj) == (0, 0):
                    nc.sync.dma_start(dbg["bcw"][:], bcW[:])
                    bcs16 = smalls.tile([128, 512], MMD, tag="bcs16")
                    nc.vector.tensor_copy(bcs16[:], bc_sb[:])
                    nc.sync.dma_start(dbg["bcsb"][:], bcs16[:])
                ct_qg = ct_tiles[qg]
                nc.vector.tensor_tensor(
                    ct_qg[:, pj, :], cxs_box[0][:], bc_sb[:], ALU.mult)

            def b_out_unit(qg, ns, oh):
                nt_i = qg * 4 + ns
                nsl = slice(ns * 128, (ns + 1) * 128)
                ct_qg = ct_tiles[qg]
                po = scps.tile([128, 2, 512], F32, tag="sc", name="po")
                for j in range(4):
                    nc.tensor.matmul(
                        po[:, 0, :],
                        ct_qg[:, j, nsl],
                        woT_sb[:, j, oh * 512:(oh + 1) * 512],
                        start=(j == 0), stop=(j == 3))
                ot = ostage.tile([128, 512], F32, tag="ot")
                nc.vector.tensor_copy(ot[:], po[:, 0, :])
                nc.sync.dma_start(
                    pout_d[nt_i * 128:(nt_i + 1) * 128,
                           oh * 512:(oh + 1) * 512],
                    ot[:])

            def b_qg_items(qg):
                prefetch_mask(qg)
                kcmax = kc_count(qg)
                items = []

                def start_qg(qg=qg):
                    ct_tiles[qg] = ctpool.tile([128, 4, 512], MMD, tag="ct",
                                               name=f"ct{qg}")
                items.append((0, start_qg))
                pending_norm = None
                for pj in range(4):
                    ctx_box = {}

                    def start_pj(ctx_box=ctx_box, qg=qg, pj=pj):
                        if qg == 3 and pj % 2 == 1:
                            # A-phase PSUM banks are free by now; use them
                            # as a second ctx buffer to pipeline pj's.
                            ca = pps.tile([128, 512], F32, tag="pp",
                                          name=f"cxa{qg}_{pj}")
                            cb = pps.tile([128, 512], F32, tag="pp",
                                          name=f"cxb{qg}_{pj}")
                            ctx_box[0] = (ca, cb)
                        else:
                            ctx_box[0] = cxps.tile([128, 2, 512], F32,
                                                   tag="cx",
                                                   name=f"cx{qg}_{pj}")
                    items.append((0, start_pj))
                    for kc in range(kcmax):
                        items.append((900,
                                      lambda qg=qg, pj=pj, kc=kc,
                                      ctx_box=ctx_box, kcmax=kcmax:
                                      b_kc_unit(qg, pj, kc, ctx_box[0],
                                                kcmax)))
                        # the deferred norm of the previous pj goes a few kc
                        # units in, so its bc matmul never heads the PE FIFO
                        # while its DVE reciprocal chain is still running.
                        if kc == 2 and pending_norm is not None:
                            items.append(pending_norm)
                            pending_norm = None
                    cxs_box = {}
                    items.append((100,
                                  lambda qg=qg, pj=pj, ctx_box=ctx_box,
                                  cxs_box=cxs_box:
                                  b_evict_unit(qg, pj, ctx_box[0], cxs_box)))
                    pending_norm = (300,
                                    lambda qg=qg, pj=pj, cxs_box=cxs_box:
                                    b_norm_unit(qg, pj, cxs_box))
                items.append(("PENDING_NORM", pending_norm))
                if DEBUG:
                    def dump_ct(qg=qg):
                        nc.sync.dma_start(dbg["ct"][:, qg, :, :],
                                          ct_tiles[qg][:])
                    items.append((0, dump_ct))
                return items

            def b_out_items(qg):
                items = []
                for ns in range(4):
                    for oh in range(2):
                        items.append((900,
                                      lambda qg=qg, ns=ns, oh=oh:
                                      b_out_unit(qg, ns, oh)))
                return items

            # ---------------- interleaved schedule ----------------
            def interleave(a_items, b_items):
                ta = sum(c for c, _ in a_items) or 1
                tb = sum(c for c, _ in b_items) or 1
                ca = cb = 0
                ia = ib = 0
                while ia < len(a_items) or ib < len(b_items):
                    if ib >= len(b_items) or (
                            ia < len(a_items) and ca * tb <= cb * ta):
                        c, f = a_items[ia]
                        ia += 1
                        ca += c
                    else:
                        c, f = b_items[ib]
                        ib += 1
                        cb += c
                    f()

            regions = [
                ([("v", 0), ("k", 0), ("q", 0)], []),
                ([("v", 1), ("k", 1), ("q", 1)], [0]),
                ([("v", 2), ("k", 2), ("q", 2)], [1]),
                ([("v", 3), ("k", 3), ("q", 3)], [2]),
                ([], [3]),
            ]
            # prefetch x for the first region up front
            for s, t in regions[0][0]:
                prefetch_x(s, t)
            held_outs = []
            for ri, (aunits, bqgs) in enumerate(regions):
                # prefetch next region's x tiles (overlaps this region)
                if ri + 1 < len(regions):
                    for s, t in regions[ri + 1][0]:
                        prefetch_x(s, t)
                a_items = [it for (s, t) in aunits for it in a_unit_items(s, t)]
                b_items = []
                for qg in bqgs:
                    flt = []
                    norm_final = None
                    for it in b_qg_items(qg):
                        if isinstance(it[0], str):
                            norm_final = it[1]
                        else:
                            flt.append(it)
                    outs = b_out_items(qg)
                    if qg == 2:
                        # hold two qg2 out-units back as tail filler: they
                        # keep the PE fed while qg3's last norm chain runs.
                        held_outs = outs[6:]
                        outs = outs[:6]
                    if qg == 3:
                        flt += held_outs
                    flt.append(norm_final)
                    b_items += flt + outs
                interleave(a_items, b_items)
            if DEBUG:
                nc.sync.dma_start(dbg["qt"][:], qt_sb[:])
                nc.sync.dma_start(dbg["kt0"][:], ktz[0][:])
                nc.sync.dma_start(dbg["kt1"][:], ktz[1][:])
                nc.sync.dma_start(dbg["v"][:], v_sb[:])
    nc.compile()
    return nc


def _host_prepare(inputs):
    """Split the full problem into 8 per-core input maps + host-side info."""
    q = np.asarray(inputs["query"], dtype=np.float32)
    k = np.asarray(inputs["key"], dtype=np.float32)
    v = np.asarray(inputs["value"], dtype=np.float32)
    mask = np.asarray(inputs["mask"])
    w = {n: np.asarray(inputs[n], dtype=np.float32)
         for n in ("wq1", "wq2", "wk1", "wk2", "wv1", "wv2", "wo")}
    bias = {n: np.asarray(inputs[n], dtype=np.float32)
            for n in ("bq1", "bq2", "bk1", "bk2", "bv1", "bv2", "bo")}

    m = mask.reshape(S, S)
    if np.array_equal(m != 0, np.tril(np.ones((S, S), bool))):
        mask_mode = "causal"
    elif np.all(m != 0):
        mask_mode = "full"
    else:
        mask_mode = "general"

    pat = None
    m01T = None
    if mask_mode == "causal":
        kk = np.arange(128)[:, None]
        qq = np.arange(512)[None, :]
        pat = np.stack(
            [(kk + 128 * i <= qq).astype(np.float32) for i in range(4)], axis=1
        )  # [128, 4, 512]
        pat = np.ascontiguousarray(pat)
    elif mask_mode == "general":
        m01T = np.ascontiguousarray((m != 0).T.astype(np.float32))

    scale = 1.0 / np.sqrt(DK).astype(np.float32)

    if MM_DTYPE == "bf16":
        import ml_dtypes

        mmd_np = ml_dtypes.bfloat16
    else:
        mmd_np = np.float32

    def cvt(a):
        return np.ascontiguousarray(a).astype(mmd_np)

    in_maps = []
    for c in range(NCORES):
        b, g = divmod(c, 2)
        sl = slice(g * GCH, (g + 1) * GCH)
        im = {
            "xqT": cvt(q[b].T),
            "xkT": cvt(k[b].T),
            "xvT": cvt(v[b].T),
            "w1T_q": cvt(w["wq1"][sl].T),
            # fold the 1/sqrt(dk) score scale into the non-silu Q branch,
            # and 0.5 everywhere (silu computed as A*(1+tanh(A/2)) = 2*silu)
            "w2T_q": cvt(w["wq2"][sl].T * (scale * 0.5)),
            "w2T_k": cvt(w["wk2"][sl].T * 0.5),
            "w2T_v": cvt(w["wv2"][sl].T * 0.5),
            "w1T_k": cvt(w["wk1"][sl].T),
            "w1T_v": cvt(w["wv1"][sl].T),
            "b1_q": np.ascontiguousarray(bias["bq1"][sl].reshape(4, 128).T),
            "b1h_q": np.ascontiguousarray(
                (bias["bq1"][sl] * 0.5).reshape(4, 128).T),
            "b2_q": np.ascontiguousarray(
                (bias["bq2"][sl] * (scale * 0.5)).reshape(4, 128).T),
            "b1_k": np.ascontiguousarray(bias["bk1"][sl].reshape(4, 128).T),
            "b1h_k": np.ascontiguousarray(
                (bias["bk1"][sl] * 0.5).reshape(4, 128).T),
            "b2_k": np.ascontiguousarray(
                (bias["bk2"][sl] * 0.5).reshape(4, 128).T),
            "b1_v": cvt(bias["bv1"][sl].reshape(1, GCH)),
            "b2_v": cvt((bias["bv2"][sl] * 0.5).reshape(1, GCH)),
            "woT": cvt(
                w["wo"][:, sl].T.reshape(4, 128, D).transpose(1, 0, 2)),
        }
        if mask_mode == "causal":
            im["pat"] = cvt(pat)
        elif mask_mode == "general":
            im["m01T"] = cvt(m01T)
        in_maps.append(im)
    return mask_mode, in_maps, bias["bo"]


def kernel(**inputs):
    global LAST_RESULT
    mask_mode, in_maps, bo = _host_prepare(inputs)
    nc = build_program(mask_mode)

    import concourse.bass_utils as bu

    if TRACE:
        import types

        try:
            from trn_agent_boot.trn_boot import _ntff_profile_via_ctypes

            hook = _ntff_profile_via_ctypes("/opt/axon/libaxon_pjrt.so")
            m = types.ModuleType("antenv.axon_hooks")
            m.get_axon_ntff_profile_hook = lambda: hook
            import antenv  # noqa: F401

            sys.modules["antenv.axon_hooks"] = m
            bu.upload_artifacts = lambda d: "local://skipped"
        except Exception as e:
            print("profiling hook install failed:", e)

    res = bu.run_bass_kernel_spmd(
        nc, in_maps, core_ids=list(range(NCORES)),
        trace=TRACE, trace_cores=TRACE_CORES,
    )
    LAST_RESULT = res

    out = np.empty((B, S, D), dtype=np.float32)
    for b in range(B):
        out[b] = (res.results[2 * b]["pout"] + res.results[2 * b + 1]["pout"]
                  + bo[None, :])
    return out
